# revision 1
# baseline (speedup 1.0000x reference)
"""Cross-cryptocurrency attention kernel for 8 Trainium2 NeuronCores.

Sharding: 16 (batch, seq-quarter) shards -> core c handles b = c//4,
query rows s in [512*(c%4), 512*(c%4+1)).  Each core computes all 8 heads
and all 9 (query-asset, key-asset) pairs for its query slice, with full
keys/values (S=2048) for its batch, so the output projection is local and
no collectives are needed.

Pipeline per (head, i, j):
  scores^T[t,s] on PE (K=hd=32, lhsT=k^T, rhs=q^T, bf16)
  exp on ACT (PSUM->SBUF bf16, scale=1/sqrt(hd))  <- bottleneck engine
  AV on PE: lhsT=[v|1] (ones column yields row-sums), rhs=E, accumulate PSUM
  normalize: PE-transpose O^T -> natural, DVE reciprocal + per-partition mul
"""

import math
import numpy as np

B = 2
S = 2048
D = 256
H = 8
HD = 32
SQ = 512  # query rows per core
N_CORES = 8
SCALE = 1.0 / math.sqrt(HD)

_CACHE = {}


def _build():
    from contextlib import ExitStack

    import concourse.bass as bass
    import concourse.mybir as mybir
    import concourse.tile as tile
    from concourse import bacc
    from concourse.masks import make_identity

    f32 = mybir.dt.float32
    bf16 = mybir.dt.bfloat16
    AF = mybir.ActivationFunctionType

    nc = bacc.Bacc("TRN2", target_bir_lowering=False, debug=False)

    x_d = nc.dram_tensor("x", [3, S, D], f32, kind="ExternalInput").ap()
    Wq_d = nc.dram_tensor("Wq", [3, D, D], f32, kind="ExternalInput").ap()
    bq_d = nc.dram_tensor("bq", [3, D], f32, kind="ExternalInput").ap()
    Wk_d = nc.dram_tensor("Wk", [3, D, D], f32, kind="ExternalInput").ap()
    bk_d = nc.dram_tensor("bk", [3, D], f32, kind="ExternalInput").ap()
    Wv_d = nc.dram_tensor("Wv", [3, D, D], f32, kind="ExternalInput").ap()
    bv_d = nc.dram_tensor("bv", [3, D], f32, kind="ExternalInput").ap()
    Wo_d = nc.dram_tensor("Wo", [D, D], f32, kind="ExternalInput").ap()
    bo_d = nc.dram_tensor("bo", [D], f32, kind="ExternalInput").ap()
    out_d = nc.dram_tensor("out", [3, SQ, D], f32, kind="ExternalOutput").ap()

    with tile.TileContext(nc) as tc, ExitStack() as ctx:
        # ---- persistent SBUF pools (bufs=1 == plain buffers) ----
        const_p = ctx.enter_context(tc.tile_pool(name="const", bufs=1))
        xT_p = ctx.enter_context(tc.tile_pool(name="xT", bufs=1))
        qkv_p = ctx.enter_context(tc.tile_pool(name="qkv", bufs=1))
        acc_p = ctx.enter_context(tc.tile_pool(name="acc", bufs=1))
        # streaming pools
        xn_p = ctx.enter_context(tc.tile_pool(name="xn", bufs=2))
        e_p = ctx.enter_context(tc.tile_pool(name="epool", bufs=4))
        sm_p = ctx.enter_context(tc.tile_pool(name="small", bufs=2))
        # PSUM: 6 + 1 + 1 = 8 banks
        ps_S = ctx.enter_context(tc.tile_pool(name="psS", bufs=2, space="PSUM"))
        ps_O = ctx.enter_context(tc.tile_pool(name="psO", bufs=1, space="PSUM"))
        ps_N = ctx.enter_context(tc.tile_pool(name="psN", bufs=1, space="PSUM"))

        # ---- constants / weights to SBUF ----
        ident = const_p.tile([128, 128], f32)
        make_identity(nc, ident[:])
        ones = const_p.tile([1, 128], f32)
        nc.gpsimd.memset(ones[:], 1.0)

        wq_sb = const_p.tile([128, 3 * 2 * D], f32)
        wk_sb = const_p.tile([128, 3 * 2 * D], f32)
        wv_sb = const_p.tile([128, 3 * 2 * D], f32)
        for w_sb, w_d in ((wq_sb, Wq_d), (wk_sb, Wk_d), (wv_sb, Wv_d)):
            nc.sync.dma_start(
                w_sb[:].rearrange("p (a kt f) -> p a kt f", a=3, kt=2),
                w_d.rearrange("a (kt p) f -> p a kt f", p=128),
            )
        wo_sb = const_p.tile([128, 2 * D], f32)
        nc.sync.dma_start(
            wo_sb[:].rearrange("p (kt f) -> p kt f", kt=2),
            Wo_d.rearrange("(kt p) f -> p kt f", p=128),
        )
        bqk_sb = const_p.tile([128, 12], f32)  # col = ty*6 + a*2 + dt (ty: q=0,k=1)
        nc.sync.dma_start(
            bqk_sb[:, 0:6].rearrange("p (a dt) -> p a dt", a=3),
            bq_d.rearrange("a (dt p) -> p a dt", p=128),
        )
        nc.sync.dma_start(
            bqk_sb[:, 6:12].rearrange("p (a dt) -> p a dt", a=3),
            bk_d.rearrange("a (dt p) -> p a dt", p=128),
        )
        bv_row = const_p.tile([1, 3 * D], f32)
        nc.sync.dma_start(bv_row[:], bv_d.rearrange("a f -> (a f)")[None, :])
        bo_row = const_p.tile([1, D], f32)
        nc.sync.dma_start(bo_row[:], bo_d[None, :])

        # ---- per-asset persistent tensors ----
        xT = [xT_p.tile([128, 2 * S], f32, tag=f"xT{_}", name=f"xT{_}") for _ in range(3)]
        kT = [qkv_p.tile([128, 2 * S], bf16, tag=f"kT{_}", name=f"kT{_}") for _ in range(3)]
        qT = [qkv_p.tile([128, 2 * SQ], bf16, tag=f"qT{_}", name=f"qT{_}") for _ in range(3)]
        v1 = [qkv_p.tile([128, 16 * (H * 33)], bf16, tag=f"v1_{_}", name=f"v1_{_}") for _ in range(3)]
        out_acc = [acc_p.tile([128, 4 * D], f32, tag=f"oacc{_}", name=f"oacc{_}") for _ in range(3)]

        # ======== Phase 1: load x, transpose, project q/k/v ========
        for a in range(3):
            xn = xn_p.tile([128, 16 * D], f32)
            for c in range(4):
                nc.sync.dma_start(
                    xn[:, c * 4 * D : (c + 1) * 4 * D].rearrange(
                        "p (st d) -> p st d", st=4
                    ),
                    x_d[a].rearrange("(st p) d -> p st d", p=128)[:, 4 * c : 4 * c + 4],
                )
            # transpose x -> xT  (16 s-tiles x 2 d-tiles)
            for dt in range(2):
                for g in range(4):  # groups of 4 s-tiles per PSUM bank
                    pst = ps_S.tile([128, 512], f32, tag="psS", name="ps1")
                    for u in range(4):
                        st = 4 * g + u
                        nc.tensor.matmul(
                            pst[:, u * 128 : (u + 1) * 128],
                            xn[:, st * D + dt * 128 : st * D + dt * 128 + 128],
                            ident[:],
                            start=True,
                            stop=True,
                        )
                    nc.vector.tensor_copy(
                        xT[a][:, dt * S + g * 512 : dt * S + (g + 1) * 512], pst[:]
                    )
            # k^T projection: psum[dout(128), t(512)] ; +bias ; -> bf16
            for dt in range(2):
                for tc4 in range(4):
                    psk = ps_S.tile([128, 512], f32, tag="psS", name="ps1")
                    for kt in range(2):
                        nc.tensor.matmul(
                            psk[:],
                            wk_sb[:, a * 2 * D + kt * D + dt * 128 : a * 2 * D + kt * D + dt * 128 + 128],
                            xT[a][:, kt * S + tc4 * 512 : kt * S + (tc4 + 1) * 512],
                            start=(kt == 0),
                            stop=(kt == 1),
                        )
                    nc.vector.tensor_scalar_add(
                        kT[a][:, dt * S + tc4 * 512 : dt * S + (tc4 + 1) * 512],
                        psk[:],
                        bqk_sb[:, 6 + a * 2 + dt : 7 + a * 2 + dt],
                    )
            # q^T projection for this core's query slice (dynamic col offset)
            for dt in range(2):
                psq = ps_S.tile([128, 512], f32, tag="psS", name="ps1")
                for kt in range(2):
                    nc.tensor.matmul(
                        psq[:],
                        wq_sb[:, a * 2 * D + kt * D + dt * 128 : a * 2 * D + kt * D + dt * 128 + 128],
                        xT[a][:, kt * S : kt * S + SQ],
                        start=(kt == 0),
                        stop=(kt == 1),
                    )
                nc.vector.tensor_scalar_add(
                    qT[a][:, dt * SQ : (dt + 1) * SQ],
                    psq[:],
                    bqk_sb[:, a * 2 + dt : 1 + a * 2 + dt],
                )
            # v projection: psum[s(128), dout(256)] ; bias via ones-row matmul
            nc.gpsimd.memset(
                v1[a].rearrange("p (t h x) -> p (t h) x", h=H, x=33)[:, :, 32:33],
                1.0,
            )
            for st in range(16):
                psv = ps_O.tile([128, D], f32, tag="psO", name="psv")
                for kt in range(2):
                    nc.tensor.matmul(
                        psv[:],
                        xT[a][:, kt * S + st * 128 : kt * S + (st + 1) * 128],
                        wv_sb[:, a * 2 * D + kt * D : a * 2 * D + (kt + 1) * D],
                        start=(kt == 0),
                        stop=False,
                    )
                nc.tensor.matmul(
                    psv[:],
                    ones[0:1, 0:128],
                    bv_row[0:1, a * D : (a + 1) * D],
                    start=False,
                    stop=True,
                )
                dst = v1[a][
                    :, st * (H * 33) : (st + 1) * (H * 33)
                ].rearrange("p (h x) -> p h x", x=33)[:, :, 0:32]
                nc.vector.tensor_copy(
                    dst, psv[:].rearrange("p (h x) -> p h x", x=32)
                )

        # ======== Phase 2: attention ========
        GROUPS = [(0, 3), (3, 3), (6, 3), (9, 3), (12, 3), (15, 1)]
        for h in range(H):
            hp = 32 * (h % 4)  # partition base for this head
            hc = h // 4  # d-tile containing this head
            for i in range(3):
                for j in range(3):
                    psO = ps_O.tile([33, 512], f32, tag="psO", name="psO")
                    for t0, glen in GROUPS:
                        psS = ps_S.tile([128, glen * 512], f32, tag="psS")
                        for u in range(glen):
                            tt = t0 + u
                            nc.tensor.matmul(
                                psS[:, u * 512 : (u + 1) * 512],
                                kT[j][hp : hp + 32, hc * S + tt * 128 : hc * S + (tt + 1) * 128],
                                qT[i][hp : hp + 32, hc * SQ : (hc + 1) * SQ],
                                start=True,
                                stop=True,
                                tile_position=(hp, 0),
                            )
                        eg = e_p.tile([128, 3 * 512], bf16, tag="eg")
                        nc.scalar.activation(
                            eg[:, 0 : glen * 512], psS[:], AF.Exp, scale=SCALE
                        )
                        for u in range(glen):
                            tt = t0 + u
                            nc.tensor.matmul(
                                psO[:],
                                v1[j][:, tt * (H * 33) + h * 33 : tt * (H * 33) + (h + 1) * 33],
                                eg[:, u * 512 : (u + 1) * 512],
                                start=(tt == 0),
                                stop=(tt == 15),
                            )
                    # normalize: copy to SBUF, PE-transpose to natural [s, d+1]
                    osb = sm_p.tile([33, 512], f32, tag="osb")
                    nc.vector.tensor_copy(osb[:], psO[:])
                    psN = ps_N.tile([128, 4 * 34], f32, tag="psN")
                    for k in range(4):
                        nc.tensor.matmul(
                            psN[:, k * 34 : k * 34 + 33],
                            osb[0:33, k * 128 : (k + 1) * 128],
                            ident[0:33, 0:33],
                            start=True,
                            stop=True,
                        )
                    rr = sm_p.tile([128, 4], f32, tag="rr")
                    nc.vector.reciprocal_approx_fast(
                        rr[:],
                        psN[:].rearrange("p (k x) -> p k x", x=34)[:, :, 32],
                    )
                    oa_view = out_acc[i].rearrange("p (k d) -> p k d", d=D)[
                        :, :, h * 32 : (h + 1) * 32
                    ]
                    if j == 0:
                        for k in range(4):
                            nc.vector.tensor_scalar_mul(
                                oa_view[:, k, :],
                                psN[:, k * 34 : k * 34 + 32],
                                rr[:, k : k + 1],
                            )
                    else:
                        tmp = sm_p.tile([128, 4 * 32], f32, tag="tmp")
                        for k in range(4):
                            nc.vector.tensor_scalar_mul(
                                tmp[:, k * 32 : (k + 1) * 32],
                                psN[:, k * 34 : k * 34 + 32],
                                rr[:, k : k + 1],
                            )
                        nc.vector.tensor_add(
                            oa_view,
                            oa_view,
                            tmp[:].rearrange("p (k d) -> p k d", d=32),
                        )

        # ======== Phase 3: output projection ========
        for a in range(3):
            aT = acc_p.tile([128, 2 * SQ], f32, tag="aT")
            for dt in range(2):
                pst = ps_S.tile([128, 512], f32, tag="psS", name="ps3")
                for st in range(4):
                    nc.tensor.matmul(
                        pst[:, st * 128 : (st + 1) * 128],
                        out_acc[a][:, st * D + dt * 128 : st * D + dt * 128 + 128],
                        ident[:],
                        start=True,
                        stop=True,
                    )
                nc.vector.tensor_copy(aT[:, dt * SQ : (dt + 1) * SQ], pst[:])
            for st in range(4):
                psf = ps_O.tile([128, D], f32, tag="psO", name="psf")
                for dt in range(2):
                    nc.tensor.matmul(
                        psf[:],
                        aT[:, dt * SQ + st * 128 : dt * SQ + (st + 1) * 128],
                        wo_sb[:, dt * D : (dt + 1) * D],
                        start=(dt == 0),
                        stop=False,
                    )
                nc.tensor.matmul(
                    psf[:],
                    ones[0:1, 0:128],
                    bo_row[0:1, :],
                    start=False,
                    stop=True,
                )
                ot = sm_p.tile([128, D], f32, tag="ot")
                nc.vector.tensor_copy(ot[:], psf[:])
                nc.sync.dma_start(
                    out_d[a].rearrange("(st p) d -> st p d", p=128)[st], ot[:]
                )

    nc.compile()
    return nc


def kernel(x_btc, x_eth, x_sol, Wq, bq, Wk, bk, Wv, bv, Wo, bo):
    from concourse.bass_utils import run_bass_kernel_spmd

    if "nc" not in _CACHE:
        _CACHE["nc"] = _build()
    nc = _CACHE["nc"]

    xs = [np.ascontiguousarray(np.asarray(t, dtype=np.float32)) for t in (x_btc, x_eth, x_sol)]
    common = {
        "Wq": np.asarray(Wq, np.float32), "bq": np.asarray(bq, np.float32),
        "Wk": np.asarray(Wk, np.float32), "bk": np.asarray(bk, np.float32),
        "Wv": np.asarray(Wv, np.float32), "bv": np.asarray(bv, np.float32),
        "Wo": np.asarray(Wo, np.float32), "bo": np.asarray(bo, np.float32),
    }
    in_maps = []
    for c in range(N_CORES):
        b, sq = c // 4, c % 4
        # Roll the sequence so this core's query quarter sits at rows [0:512)
        # (the kernel always projects q from rows 0:512).  k/v see the rolled
        # full sequence, which is fine: softmax+sum over the key axis is
        # permutation-invariant.
        xq = np.stack(
            [np.roll(xs[i][b], -sq * SQ, axis=0) for i in range(3)]
        ).astype(np.float32)
        in_maps.append({"x": np.ascontiguousarray(xq), **common})
    import os
    res = run_bass_kernel_spmd(
        nc, in_maps, core_ids=list(range(N_CORES)),
        trace=bool(os.environ.get("BASS_TRACE")),
    )
    _CACHE["last_res"] = res

    outs = [np.empty((B, S, D), np.float32) for _ in range(3)]
    for c in range(N_CORES):
        b, sq = c // 4, c % 4
        o = res.results[c]["out"]
        for i in range(3):
            outs[i][b, sq * SQ : (sq + 1) * SQ] = o[i]
    return tuple(outs)


if __name__ == "__main__":
    import reference

    inp = reference.setup_inputs()
    inp = {k: np.asarray(v) for k, v in inp.items()}
    got = kernel(**inp)
    exp = reference.reference(**inp)
    for i in range(3):
        g, e = np.asarray(got[i]), np.asarray(exp[i])
        err = np.abs(g - e).max() / np.abs(e).max()
        print(f"out[{i}] rel err {err:.3e}")



# revision 4
# speedup vs baseline: 1.2856x; 1.2856x over previous
"""Cross-cryptocurrency attention kernel for 8 Trainium2 NeuronCores.

Sharding: 16 (batch, seq-quarter) shards -> core c handles b = c//4,
query rows s in [512*(c%4), 512*(c%4+1)).  Each core computes all 8 heads
and all 9 (query-asset, key-asset) pairs for its query slice, with full
keys/values (S=2048) for its batch, so the output projection is local and
no collectives are needed.

v2 design (ACT exp is the hard floor; everything else tucks under it):
  - host folds biases: bk drops out of softmax exactly; bv folds into
    bo2 = bo + (sum_j bv_j) @ Wo; x/weights pre-cast to bf16.
  - projections all-bf16 (1 PE cycle/row instead of 4 for f32).
  - scores^T[t,s] on PE (lhsT=k^T bf16), exp on ACT (PSUM->SBUF bf16).
  - AV with the *E tile* as the stationary operand: out[s,33] accumulates
    over 16 t-tiles at 33 rows each (vs 512) -- 4x less PE time; the
    ones column of [v|1] yields row-sums Z in col 32.
  - normalize straight out of the AV psum: DVE reciprocal of Z +
    scalar_tensor_tensor (mul rr, add acc) -- no PE transpose needed.
  - software pipeline: per score group g: PE scores(g) -> ACT exp(g) ->
    PE AV(g-1); asset-1/2 projections drip between early combos so ACT
    never starves.
"""

import math
import numpy as np

B = 2
S = 2048
D = 256
H = 8
HD = 32
SQ = 512  # query rows per core
N_CORES = 8
SCALE = 1.0 / math.sqrt(HD)
GROUPS = [(0, 3), (3, 3), (6, 3), (9, 3), (12, 3), (15, 1)]

_CACHE = {}


def _build():
    from contextlib import ExitStack

    import concourse.bass as bass
    import concourse.mybir as mybir
    import concourse.tile as tile
    from concourse import bacc
    from concourse.masks import make_identity

    f32 = mybir.dt.float32
    bf16 = mybir.dt.bfloat16
    AF = mybir.ActivationFunctionType
    ALU = mybir.AluOpType

    nc = bacc.Bacc("TRN2", target_bir_lowering=False, debug=False)

    x_d = nc.dram_tensor("x", [3, S, D], bf16, kind="ExternalInput").ap()
    Wq_d = nc.dram_tensor("Wq", [3, D, D], bf16, kind="ExternalInput").ap()
    bq_d = nc.dram_tensor("bq", [3, D], f32, kind="ExternalInput").ap()
    Wk_d = nc.dram_tensor("Wk", [3, D, D], bf16, kind="ExternalInput").ap()
    Wv_d = nc.dram_tensor("Wv", [3, D, D], bf16, kind="ExternalInput").ap()
    Wo_d = nc.dram_tensor("Wo", [D, D], bf16, kind="ExternalInput").ap()
    bo2_d = nc.dram_tensor("bo2", [D], bf16, kind="ExternalInput").ap()
    out_d = nc.dram_tensor("out", [3, SQ, D], f32, kind="ExternalOutput").ap()

    with tile.TileContext(nc) as tc, ExitStack() as ctx:
        const_p = ctx.enter_context(tc.tile_pool(name="const", bufs=1))
        xT_p = ctx.enter_context(tc.tile_pool(name="xT", bufs=1))
        qkv_p = ctx.enter_context(tc.tile_pool(name="qkv", bufs=1))
        acc_p = ctx.enter_context(tc.tile_pool(name="acc", bufs=1))
        xn_p = ctx.enter_context(tc.tile_pool(name="xn", bufs=2))
        e_p = ctx.enter_context(tc.tile_pool(name="epool", bufs=4))
        sm_p = ctx.enter_context(tc.tile_pool(name="small", bufs=2))
        # PSUM: 3+3 (score groups) + 1+1 (AV accum) = 8 banks
        ps_S = ctx.enter_context(tc.tile_pool(name="psS", bufs=2, space="PSUM"))
        ps_A = ctx.enter_context(tc.tile_pool(name="psA", bufs=2, space="PSUM"))

        # ---- constants / weights to SBUF ----
        ident = const_p.tile([128, 128], f32)
        make_identity(nc, ident[:])
        identb = const_p.tile([128, 128], bf16)
        make_identity(nc, identb[:])
        onesb = const_p.tile([1, 128], bf16)
        nc.gpsimd.memset(onesb[:], 1.0)

        wq_sb = const_p.tile([128, 3 * 2 * D], bf16)
        wk_sb = const_p.tile([128, 3 * 2 * D], bf16)
        wv_sb = const_p.tile([128, 3 * 2 * D], bf16)
        for w_sb, w_d in ((wq_sb, Wq_d), (wk_sb, Wk_d), (wv_sb, Wv_d)):
            nc.sync.dma_start(
                w_sb[:].rearrange("p (a kt f) -> p a kt f", a=3, kt=2),
                w_d.rearrange("a (kt p) f -> p a kt f", p=128),
            )
        wo_sb = const_p.tile([128, 2 * D], bf16)
        nc.sync.dma_start(
            wo_sb[:].rearrange("p (kt f) -> p kt f", kt=2),
            Wo_d.rearrange("(kt p) f -> p kt f", p=128),
        )
        bq_sb = const_p.tile([128, 6], f32)  # col = a*2 + dt
        nc.sync.dma_start(
            bq_sb[:].rearrange("p (a dt) -> p a dt", a=3),
            bq_d.rearrange("a (dt p) -> p a dt", p=128),
        )
        bo2_row = const_p.tile([1, D], bf16)
        nc.sync.dma_start(bo2_row[:], bo2_d[None, :])

        # ---- per-asset persistent tensors ----
        xT = [xT_p.tile([128, 2 * S], bf16, tag=f"xT{_}", name=f"xT{_}") for _ in range(3)]
        kT = [qkv_p.tile([128, 2 * S], bf16, tag=f"kT{_}", name=f"kT{_}") for _ in range(3)]
        qT = [qkv_p.tile([128, 2 * SQ], bf16, tag=f"qT{_}", name=f"qT{_}") for _ in range(3)]
        v1 = [qkv_p.tile([128, 16 * (H * 33)], bf16, tag=f"v1_{_}", name=f"v1_{_}") for _ in range(3)]
        out_acc = [acc_p.tile([128, 4 * D], f32, tag=f"oacc{_}", name=f"oacc{_}") for _ in range(3)]

        # ======== Phase 1 as a unit generator (dripped between combos) ====
        def proj_units(a):
            xn = xn_p.tile([128, 16 * D], bf16, tag="xn", name=f"xn{a}")

            def dma_unit(c):
                def run():
                    nc.sync.dma_start(
                        xn[:, c * 4 * D : (c + 1) * 4 * D].rearrange(
                            "p (st d) -> p st d", st=4
                        ),
                        x_d[a].rearrange("(st p) d -> p st d", p=128)[:, 4 * c : 4 * c + 4],
                    )
                return run

            for c in range(4):
                yield dma_unit(c)

            def transp_unit(dt, g):
                def run():
                    pst = ps_S.tile([128, 512], bf16, tag="psS", name="ps1")
                    for u in range(4):
                        st = 4 * g + u
                        nc.tensor.transpose(
                            pst[:, u * 128 : (u + 1) * 128],
                            xn[:, st * D + dt * 128 : st * D + dt * 128 + 128],
                            identb[:],
                        )
                    nc.vector.tensor_copy(
                        xT[a][:, dt * S + g * 512 : dt * S + (g + 1) * 512], pst[:]
                    )
                return run

            for dt in range(2):
                for g in range(4):
                    yield transp_unit(dt, g)

            # k^T projection (bias bk dropped: softmax shift-invariant)
            def k_unit(dt, tc4):
                def run():
                    psk = ps_S.tile([128, 512], f32, tag="psS", name="ps1")
                    for kt in range(2):
                        nc.tensor.matmul(
                            psk[:],
                            wk_sb[:, a * 2 * D + kt * D + dt * 128 : a * 2 * D + kt * D + dt * 128 + 128],
                            xT[a][:, kt * S + tc4 * 512 : kt * S + (tc4 + 1) * 512],
                            start=(kt == 0),
                            stop=(kt == 1),
                        )
                    nc.vector.tensor_copy(
                        kT[a][:, dt * S + tc4 * 512 : dt * S + (tc4 + 1) * 512], psk[:]
                    )
                return run

            # q^T projection (+ bq)
            def q_unit(dt):
                def run():
                    psq = ps_S.tile([128, 512], f32, tag="psS", name="ps1")
                    for kt in range(2):
                        nc.tensor.matmul(
                            psq[:],
                            wq_sb[:, a * 2 * D + kt * D + dt * 128 : a * 2 * D + kt * D + dt * 128 + 128],
                            xT[a][:, kt * S : kt * S + SQ],
                            start=(kt == 0),
                            stop=(kt == 1),
                        )
                    nc.vector.tensor_scalar_add(
                        qT[a][:, dt * SQ : (dt + 1) * SQ],
                        psq[:],
                        bq_sb[:, a * 2 + dt : 1 + a * 2 + dt],
                    )
                return run

            for dt in range(2):
                yield q_unit(dt)
            for dt in range(2):
                for tc4 in range(4):
                    yield k_unit(dt, tc4)

            # v projection (bias bv folded into bo2 host-side)
            def ones_unit():
                def run():
                    nc.gpsimd.memset(
                        v1[a].rearrange("p (t h x) -> p (t h) x", h=H, x=33)[:, :, 32:33],
                        1.0,
                    )
                return run

            yield ones_unit()

            def v_unit(st):
                def run():
                    psv = ps_A.tile([128, D], f32, tag="psA", name="psv")
                    for kt in range(2):
                        nc.tensor.matmul(
                            psv[:],
                            xT[a][:, kt * S + st * 128 : kt * S + (st + 1) * 128],
                            wv_sb[:, a * 2 * D + kt * D : a * 2 * D + (kt + 1) * D],
                            start=(kt == 0),
                            stop=(kt == 1),
                        )
                    dst = v1[a][
                        :, st * (H * 33) : (st + 1) * (H * 33)
                    ].rearrange("p (h x) -> p h x", x=33)[:, :, 0:32]
                    nc.vector.tensor_copy(
                        dst, psv[:].rearrange("p (h x) -> p h x", x=32)
                    )
                return run

            for st in range(16):
                yield v_unit(st)

        # ======== Phase 2: one (i, j, h) combo ========
        def emit_av(eg, t0, glen, j, h, psA):
            # One accumulation group for the whole bank: start=True lazily
            # zeroes the full 2KB zero region, so only the very first matmul
            # may set it; the other three s-subtile chains accumulate onto
            # pending-zero bytes.
            for u in range(glen):
                tt = t0 + u
                for k in range(4):
                    nc.tensor.matmul(
                        psA[:, k * 33 : (k + 1) * 33],
                        eg[:, u * 512 + k * 128 : u * 512 + (k + 1) * 128],
                        v1[j][:, tt * (H * 33) + h * 33 : tt * (H * 33) + (h + 1) * 33],
                        start=(tt == 0 and k == 0),
                        stop=(tt == 15 and k == 3),
                        skip_group_check=True,
                    )

        def combo(i, j, h):
            hp = 32 * (h % 4)
            hc = h // 4
            psA = ps_A.tile([128, 4 * 33], f32, tag="psA", name="psA")
            egs = []
            for gi, (t0, glen) in enumerate(GROUPS):
                psS = ps_S.tile([128, glen * 512], f32, tag="psS", name="ps2")
                for u in range(glen):
                    tt = t0 + u
                    nc.tensor.matmul(
                        psS[:, u * 512 : (u + 1) * 512],
                        kT[j][hp : hp + 32, hc * S + tt * 128 : hc * S + (tt + 1) * 128],
                        qT[i][hp : hp + 32, hc * SQ : (hc + 1) * SQ],
                        start=True,
                        stop=True,
                        tile_position=(hp, 0),
                    )
                eg = e_p.tile([128, 3 * 512], bf16, tag="eg", name="eg")
                nc.scalar.activation(eg[:, 0 : glen * 512], psS[:], AF.Exp, scale=SCALE)
                egs.append((eg, t0, glen))
                if gi >= 1:
                    emit_av(*egs[gi - 1], j, h, psA)
            emit_av(*egs[-1], j, h, psA)
            # normalize from AV psum: rr = 1/Z, acc += O * rr
            rr4 = sm_p.tile([128, 4], f32, tag="rr", name="rr")
            nc.vector.reciprocal_approx_fast(
                rr4[:],
                psA[:].rearrange("p (k x) -> p k x", x=33)[:, :, 32],
            )
            for k in range(4):
                oa = out_acc[i][:, k * D + h * 32 : k * D + (h + 1) * 32]
                src = psA[:, k * 33 : k * 33 + 32]
                if j == 0:
                    nc.vector.tensor_scalar_mul(oa, src, rr4[:, k : k + 1])
                else:
                    nc.vector.scalar_tensor_tensor(
                        oa, src, rr4[:, k : k + 1], oa, op0=ALU.mult, op1=ALU.add
                    )

        # ======== Phase 3: output projection for one asset ========
        def phase3(i):
            aT = acc_p.tile([128, 2 * SQ], bf16, tag="aT", name="aT")
            for dt in range(2):
                pst = ps_S.tile([128, 512], f32, tag="psS", name="ps3")
                for st in range(4):
                    nc.tensor.transpose(
                        pst[:, st * 128 : (st + 1) * 128],
                        out_acc[i][:, st * D + dt * 128 : st * D + dt * 128 + 128],
                        ident[:],
                    )
                nc.vector.tensor_copy(aT[:, dt * SQ : (dt + 1) * SQ], pst[:])
            for st in range(4):
                psf = ps_A.tile([128, D], f32, tag="psA", name="psf")
                for dt in range(2):
                    nc.tensor.matmul(
                        psf[:],
                        aT[:, dt * SQ + st * 128 : dt * SQ + (st + 1) * 128],
                        wo_sb[:, dt * D : (dt + 1) * D],
                        start=(dt == 0),
                        stop=False,
                    )
                nc.tensor.matmul(
                    psf[:],
                    onesb[0:1, 0:128],
                    bo2_row[0:1, :],
                    start=False,
                    stop=True,
                )
                ot = sm_p.tile([128, D], f32, tag="ot", name="ot")
                nc.vector.tensor_copy(ot[:], psf[:])
                nc.sync.dma_start(
                    out_d[i].rearrange("(st p) d -> st p d", p=128)[st], ot[:]
                )

        # ======== Emission schedule ========
        for u in proj_units(0):
            u()

        # remaining projection work, dripped between early combos so the
        # PE keeps feeding ACT without long bubbles
        drip = list(proj_units(1)) + list(proj_units(2))
        DRIP_PER_COMBO = 6

        for i in range(3):
            for j in range(3):
                for h in range(H):
                    combo(i, j, h)
                    for _ in range(DRIP_PER_COMBO):
                        if drip:
                            drip.pop(0)()
            phase3(i)
    nc.compile()
    return nc


def kernel(x_btc, x_eth, x_sol, Wq, bq, Wk, bk, Wv, bv, Wo, bo):
    import ml_dtypes
    from concourse.bass_utils import run_bass_kernel_spmd

    if "nc" not in _CACHE:
        _CACHE["nc"] = _build()
    nc = _CACHE["nc"]

    bff = ml_dtypes.bfloat16
    xs = [np.asarray(t, dtype=np.float32) for t in (x_btc, x_eth, x_sol)]
    # fold v-bias and o-bias: out = attn @ Wo + (sum_j bv_j) @ Wo + bo
    bo2 = (np.asarray(bo, np.float64)
           + np.asarray(bv, np.float64).sum(0) @ np.asarray(Wo, np.float64))
    common = {
        "Wq": np.asarray(Wq, np.float32).astype(bff),
        "bq": np.asarray(bq, np.float32),
        "Wk": np.asarray(Wk, np.float32).astype(bff),
        "Wv": np.asarray(Wv, np.float32).astype(bff),
        "Wo": np.asarray(Wo, np.float32).astype(bff),
        "bo2": bo2.astype(np.float32).astype(bff),
    }
    in_maps = []
    for c in range(N_CORES):
        b, sq = c // 4, c % 4
        # Roll the sequence so this core's query quarter sits at rows [0:512)
        # (the kernel always projects q from rows 0:512).  k/v see the rolled
        # full sequence, which is fine: softmax+sum over the key axis is
        # permutation-invariant.
        xq = np.stack(
            [np.roll(xs[i][b], -sq * SQ, axis=0) for i in range(3)]
        ).astype(bff)
        in_maps.append({"x": np.ascontiguousarray(xq), **common})
    import os
    res = run_bass_kernel_spmd(
        nc, in_maps, core_ids=list(range(N_CORES)),
        trace=bool(os.environ.get("BASS_TRACE")),
    )
    _CACHE["last_res"] = res

    outs = [np.empty((B, S, D), np.float32) for _ in range(3)]
    for c in range(N_CORES):
        b, sq = c // 4, c % 4
        o = res.results[c]["out"]
        for i in range(3):
            outs[i][b, sq * SQ : (sq + 1) * SQ] = o[i]
    return tuple(outs)


if __name__ == "__main__":
    import reference

    inp = reference.setup_inputs()
    inp = {k: np.asarray(v) for k, v in inp.items()}
    got = kernel(**inp)
    exp = reference.reference(**inp)
    for i in range(3):
        g, e = np.asarray(got[i]), np.asarray(exp[i])
        err = np.abs(g - e).max() / np.abs(e).max()
        print(f"out[{i}] rel err {err:.3e}")


# revision 7
# speedup vs baseline: 1.2866x; 1.0008x over previous
"""Cross-cryptocurrency attention kernel for 8 Trainium2 NeuronCores.

Sharding: 16 (batch, seq-quarter) shards -> core c handles b = c//4,
query rows s in [512*(c%4), 512*(c%4+1)).  Each core computes all 8 heads
and all 9 (query-asset, key-asset) pairs for its query slice, with full
keys/values (S=2048) for its batch, so the output projection is local and
no collectives are needed.

v2 design (ACT exp is the hard floor; everything else tucks under it):
  - host folds biases: bk drops out of softmax exactly; bv folds into
    bo2 = bo + (sum_j bv_j) @ Wo; x/weights pre-cast to bf16.
  - projections all-bf16 (1 PE cycle/row instead of 4 for f32).
  - scores^T[t,s] on PE (lhsT=k^T bf16), exp on ACT (PSUM->SBUF bf16).
  - AV with the *E tile* as the stationary operand: out[s,33] accumulates
    over 16 t-tiles at 33 rows each (vs 512) -- 4x less PE time; the
    ones column of [v|1] yields row-sums Z in col 32.
  - normalize straight out of the AV psum: DVE reciprocal of Z +
    scalar_tensor_tensor (mul rr, add acc) -- no PE transpose needed.
  - software pipeline: per score group g: PE scores(g) -> ACT exp(g) ->
    PE AV(g-1); asset-1/2 projections drip between early combos so ACT
    never starves.
"""

import math
import numpy as np

B = 2
S = 2048
D = 256
H = 8
HD = 32
SQ = 512  # query rows per core
N_CORES = 8
SCALE = 1.0 / math.sqrt(HD)
GROUPS = [(0, 3), (3, 3), (6, 3), (9, 3), (12, 3), (15, 1)]

_CACHE = {}


def _build():
    from contextlib import ExitStack

    import concourse.bass as bass
    import concourse.mybir as mybir
    import concourse.tile as tile
    from concourse import bacc
    from concourse.masks import make_identity

    f32 = mybir.dt.float32
    bf16 = mybir.dt.bfloat16
    AF = mybir.ActivationFunctionType
    ALU = mybir.AluOpType

    nc = bacc.Bacc("TRN2", target_bir_lowering=False, debug=False)

    x_d = nc.dram_tensor("x", [3, S, D], bf16, kind="ExternalInput").ap()
    Wq_d = nc.dram_tensor("Wq", [3, D, D], bf16, kind="ExternalInput").ap()
    bq_d = nc.dram_tensor("bq", [3, D], f32, kind="ExternalInput").ap()
    Wk_d = nc.dram_tensor("Wk", [3, D, D], bf16, kind="ExternalInput").ap()
    Wv_d = nc.dram_tensor("Wv", [3, D, D], bf16, kind="ExternalInput").ap()
    Wo_d = nc.dram_tensor("Wo", [D, D], bf16, kind="ExternalInput").ap()
    bo2_d = nc.dram_tensor("bo2", [D], bf16, kind="ExternalInput").ap()
    out_d = nc.dram_tensor("out", [3, SQ, D], f32, kind="ExternalOutput").ap()

    with tile.TileContext(nc) as tc, ExitStack() as ctx:
        const_p = ctx.enter_context(tc.tile_pool(name="const", bufs=1))
        xT_p = ctx.enter_context(tc.tile_pool(name="xT", bufs=1))
        qkv_p = ctx.enter_context(tc.tile_pool(name="qkv", bufs=1))
        acc_p = ctx.enter_context(tc.tile_pool(name="acc", bufs=1))
        xn_p = ctx.enter_context(tc.tile_pool(name="xn", bufs=2))
        e_p = ctx.enter_context(tc.tile_pool(name="epool", bufs=4))
        sm_p = ctx.enter_context(tc.tile_pool(name="small", bufs=2))
        # PSUM: 3+3 (score groups) + 1+1 (AV accum) = 8 banks
        ps_S = ctx.enter_context(tc.tile_pool(name="psS", bufs=2, space="PSUM"))
        ps_A = ctx.enter_context(tc.tile_pool(name="psA", bufs=2, space="PSUM"))

        # ---- constants / weights to SBUF ----
        ident = const_p.tile([128, 128], f32)
        make_identity(nc, ident[:])
        identb = const_p.tile([128, 128], bf16)
        make_identity(nc, identb[:])
        onesb = const_p.tile([1, 128], bf16)
        nc.gpsimd.memset(onesb[:], 1.0)

        wq_sb = const_p.tile([128, 3 * 2 * D], bf16)
        wk_sb = const_p.tile([128, 3 * 2 * D], bf16)
        wv_sb = const_p.tile([128, 3 * 2 * D], bf16)
        for w_sb, w_d in ((wq_sb, Wq_d), (wk_sb, Wk_d), (wv_sb, Wv_d)):
            nc.sync.dma_start(
                w_sb[:].rearrange("p (a kt f) -> p a kt f", a=3, kt=2),
                w_d.rearrange("a (kt p) f -> p a kt f", p=128),
            )
        wo_sb = const_p.tile([128, 2 * D], bf16)
        nc.sync.dma_start(
            wo_sb[:].rearrange("p (kt f) -> p kt f", kt=2),
            Wo_d.rearrange("(kt p) f -> p kt f", p=128),
        )
        bq_sb = const_p.tile([128, 6], f32)  # col = a*2 + dt
        nc.sync.dma_start(
            bq_sb[:].rearrange("p (a dt) -> p a dt", a=3),
            bq_d.rearrange("a (dt p) -> p a dt", p=128),
        )
        bo2_row = const_p.tile([1, D], bf16)
        nc.sync.dma_start(bo2_row[:], bo2_d[None, :])

        # ---- per-asset persistent tensors ----
        xT = [xT_p.tile([128, 2 * S], bf16, tag=f"xT{_}", name=f"xT{_}") for _ in range(3)]
        kT = [qkv_p.tile([128, 2 * S], bf16, tag=f"kT{_}", name=f"kT{_}") for _ in range(3)]
        qT = [qkv_p.tile([128, 2 * SQ], bf16, tag=f"qT{_}", name=f"qT{_}") for _ in range(3)]
        v1 = [qkv_p.tile([128, 16 * (H * 33)], bf16, tag=f"v1_{_}", name=f"v1_{_}") for _ in range(3)]
        out_acc = [acc_p.tile([128, 4 * D], f32, tag=f"oacc{_}", name=f"oacc{_}") for _ in range(3)]

        # ======== Phase 1 as a unit generator (dripped between combos) ====
        def proj_units(a):
            xn = xn_p.tile([128, 16 * D], bf16, tag="xn", name=f"xn{a}")

            def dma_unit(c):
                def run():
                    nc.sync.dma_start(
                        xn[:, c * 4 * D : (c + 1) * 4 * D].rearrange(
                            "p (st d) -> p st d", st=4
                        ),
                        x_d[a].rearrange("(st p) d -> p st d", p=128)[:, 4 * c : 4 * c + 4],
                    )
                return run

            for c in range(4):
                yield dma_unit(c)

            def transp_unit(dt, g):
                def run():
                    pst = ps_S.tile([128, 512], bf16, tag="psS", name="ps1")
                    for u in range(4):
                        st = 4 * g + u
                        nc.tensor.transpose(
                            pst[:, u * 128 : (u + 1) * 128],
                            xn[:, st * D + dt * 128 : st * D + dt * 128 + 128],
                            identb[:],
                        )
                    nc.vector.tensor_copy(
                        xT[a][:, dt * S + g * 512 : dt * S + (g + 1) * 512], pst[:]
                    )
                return run

            for dt in range(2):
                for g in range(4):
                    yield transp_unit(dt, g)

            # k^T projection (bias bk dropped: softmax shift-invariant)
            def k_unit(dt, tc4):
                def run():
                    psk = ps_S.tile([128, 512], f32, tag="psS", name="ps1")
                    for kt in range(2):
                        nc.tensor.matmul(
                            psk[:],
                            wk_sb[:, a * 2 * D + kt * D + dt * 128 : a * 2 * D + kt * D + dt * 128 + 128],
                            xT[a][:, kt * S + tc4 * 512 : kt * S + (tc4 + 1) * 512],
                            start=(kt == 0),
                            stop=(kt == 1),
                        )
                    nc.vector.tensor_copy(
                        kT[a][:, dt * S + tc4 * 512 : dt * S + (tc4 + 1) * 512], psk[:]
                    )
                return run

            # q^T projection (+ bq)
            def q_unit(dt):
                def run():
                    psq = ps_S.tile([128, 512], f32, tag="psS", name="ps1")
                    for kt in range(2):
                        nc.tensor.matmul(
                            psq[:],
                            wq_sb[:, a * 2 * D + kt * D + dt * 128 : a * 2 * D + kt * D + dt * 128 + 128],
                            xT[a][:, kt * S : kt * S + SQ],
                            start=(kt == 0),
                            stop=(kt == 1),
                        )
                    nc.vector.tensor_scalar_add(
                        qT[a][:, dt * SQ : (dt + 1) * SQ],
                        psq[:],
                        bq_sb[:, a * 2 + dt : 1 + a * 2 + dt],
                    )
                return run

            for dt in range(2):
                yield q_unit(dt)
            for dt in range(2):
                for tc4 in range(4):
                    yield k_unit(dt, tc4)

            # v projection (bias bv folded into bo2 host-side)
            def ones_unit():
                def run():
                    nc.gpsimd.memset(
                        v1[a].rearrange("p (t h x) -> p (t h) x", h=H, x=33)[:, :, 32:33],
                        1.0,
                    )
                return run

            yield ones_unit()

            # v projection: two t-tiles per unit sharing one psum bank (the
            # second chain rides the first's lazy-zeroed region)
            def v_unit(pr):
                def run():
                    psv = ps_S.tile([128, 512], f32, tag="psS", name="psv")
                    for half in range(2):
                        st = 2 * pr + half
                        for kt in range(2):
                            nc.tensor.matmul(
                                psv[:, half * D : (half + 1) * D],
                                xT[a][:, kt * S + st * 128 : kt * S + (st + 1) * 128],
                                wv_sb[:, a * 2 * D + kt * D : a * 2 * D + (kt + 1) * D],
                                start=(half == 0 and kt == 0),
                                stop=(half == 1 and kt == 1),
                                skip_group_check=True,
                            )
                    dst = v1[a][
                        :, 2 * pr * (H * 33) : (2 * pr + 2) * (H * 33)
                    ].rearrange("p (s2 h x) -> p s2 h x", s2=2, x=33)[:, :, :, 0:32]
                    nc.vector.tensor_copy(
                        dst, psv[:].rearrange("p (s2 h x) -> p s2 h x", s2=2, x=32)
                    )
                return run

            for pr in range(8):
                yield v_unit(pr)

        # ======== Phase 2: one (i, j, h) combo ========
        def emit_av(eg, t0, glen, j, h, psA):
            # One accumulation group for the whole bank: start=True lazily
            # zeroes the full 2KB zero region, so only the very first matmul
            # may set it; the other three s-subtile chains accumulate onto
            # pending-zero bytes.
            for u in range(glen):
                tt = t0 + u
                for k in range(4):
                    nc.tensor.matmul(
                        psA[:, k * 33 : (k + 1) * 33],
                        eg[:, u * 512 + k * 128 : u * 512 + (k + 1) * 128],
                        v1[j][:, tt * (H * 33) + h * 33 : tt * (H * 33) + (h + 1) * 33],
                        start=(tt == 0 and k == 0),
                        stop=(tt == 15 and k == 3),
                        skip_group_check=True,
                    )

        def norm_unit(i, j, h, psA):
            # normalize from AV psum: rr = 1/Z, acc += O * rr
            def run():
                rr4 = sm_p.tile([128, 4], f32, tag="rr", name="rr")
                nc.vector.reciprocal_approx_fast(
                    rr4[:],
                    psA[:].rearrange("p (k x) -> p k x", x=33)[:, :, 32],
                )
                for k in range(4):
                    oa = out_acc[i][:, k * D + h * 32 : k * D + (h + 1) * 32]
                    src = psA[:, k * 33 : k * 33 + 32]
                    if j == 0:
                        nc.vector.tensor_scalar_mul(oa, src, rr4[:, k : k + 1])
                    else:
                        nc.vector.scalar_tensor_tensor(
                            oa, src, rr4[:, k : k + 1], oa, op0=ALU.mult, op1=ALU.add
                        )
            return run

        def combo(i, j, h, tail, drip):
            """Emit one (i,j,h) combo.  `tail` is the deferred work of the
            previous combo (its last two AV batches + normalize), flushed
            right after this combo's first score group so the PE has already
            queued scores(g0) when the previous combo's last exp retires.
            Returns this combo's own tail.  `drip(n)` emits up to n pairs of
            background psum units at parity-neutral points."""
            hp = 32 * (h % 4)
            hc = h // 4
            psA = ps_A.tile([128, 4 * 33], f32, tag="psA", name="psA")
            egs = []

            def sc(gi):
                t0, glen = GROUPS[gi]
                psS = ps_S.tile([128, glen * 512], f32, tag="psS", name="ps2")
                for u in range(glen):
                    tt = t0 + u
                    nc.tensor.matmul(
                        psS[:, u * 512 : (u + 1) * 512],
                        kT[j][hp : hp + 32, hc * S + tt * 128 : hc * S + (tt + 1) * 128],
                        qT[i][hp : hp + 32, hc * SQ : (hc + 1) * SQ],
                        start=True,
                        stop=True,
                        tile_position=(hp, 0),
                    )
                eg = e_p.tile([128, 3 * 512], bf16, tag="eg", name="eg")
                nc.scalar.activation(eg[:, 0 : glen * 512], psS[:], AF.Exp, scale=SCALE)
                egs.append((eg, t0, glen))

            sc(0)
            for t in tail:
                t()
            sc(1)
            drip(1)
            emit_av(*egs[0], j, h, psA)
            sc(2)
            emit_av(*egs[1], j, h, psA)
            sc(3)
            drip(1)
            emit_av(*egs[2], j, h, psA)
            sc(4)
            emit_av(*egs[3], j, h, psA)
            sc(5)
            return [
                lambda: emit_av(*egs[4], j, h, psA),
                lambda: emit_av(*egs[5], j, h, psA),
                norm_unit(i, j, h, psA),
            ]

        # ======== Phase 3: output projection for one asset, as units ======
        def phase3_units(i):
            aT = acc_p.tile([128, 2 * SQ], bf16, tag=f"aT{i}", name=f"aT{i}")

            def t_unit(dt):
                def run():
                    pst = ps_S.tile([128, 512], f32, tag="psS", name="ps3")
                    for st in range(4):
                        nc.tensor.transpose(
                            pst[:, st * 128 : (st + 1) * 128],
                            out_acc[i][:, st * D + dt * 128 : st * D + dt * 128 + 128],
                            ident[:],
                        )
                    nc.vector.tensor_copy(aT[:, dt * SQ : (dt + 1) * SQ], pst[:])
                return run

            for dt in range(2):
                yield t_unit(dt)

            def p_unit(st):
                def run():
                    psf = ps_S.tile([128, D], f32, tag="psS", name="psf")
                    for dt in range(2):
                        nc.tensor.matmul(
                            psf[:],
                            aT[:, dt * SQ + st * 128 : dt * SQ + (st + 1) * 128],
                            wo_sb[:, dt * D : (dt + 1) * D],
                            start=(dt == 0),
                            stop=False,
                        )
                    nc.tensor.matmul(
                        psf[:],
                        onesb[0:1, 0:128],
                        bo2_row[0:1, :],
                        start=False,
                        stop=True,
                    )
                    ot = sm_p.tile([128, D], f32, tag="ot", name="ot")
                    nc.vector.tensor_copy(ot[:], psf[:])
                    nc.sync.dma_start(
                        out_d[i].rearrange("(st p) d -> st p d", p=128)[st], ot[:]
                    )
                return run

            for st in range(4):
                yield p_unit(st)

        # ======== Emission schedule ========
        for u in proj_units(0):
            u()

        # Deadline-tagged drip queue: (unit, not_before_combo).  Assets 1/2
        # projections must land before their first combos ((0,1,*) at 8,
        # (0,2,*) at 16 -- both met with huge slack at 4 units/combo);
        # phase3(i) units wait for asset i's last normalize (flushed at the
        # start of combo 24*(i+1)).
        dripq = []
        for a in (1, 2):
            for u in proj_units(a):
                dripq.append((u, 0))
        for idx, u in enumerate(phase3_units(0)):
            dripq.append((u, 24 + idx // 2))
        for idx, u in enumerate(phase3_units(1)):
            dripq.append((u, 48 + idx // 2))

        combo_idx = [0]

        def drip(n_pairs):
            budget = 2 * n_pairs
            while dripq and budget > 0 and dripq[0][1] <= combo_idx[0]:
                u, _ = dripq.pop(0)
                u()
                budget -= 1

        tail = []
        for i in range(3):
            for j in range(3):
                for h in range(H):
                    tail = combo(i, j, h, tail, drip)
                    combo_idx[0] += 1
        for t in tail:
            t()
        while dripq:
            dripq.pop(0)[0]()
        for u in phase3_units(2):
            u()
    nc.compile()
    return nc


def kernel(x_btc, x_eth, x_sol, Wq, bq, Wk, bk, Wv, bv, Wo, bo):
    import ml_dtypes
    from concourse.bass_utils import run_bass_kernel_spmd

    if "nc" not in _CACHE:
        _CACHE["nc"] = _build()
    nc = _CACHE["nc"]

    bff = ml_dtypes.bfloat16
    xs = [np.asarray(t, dtype=np.float32) for t in (x_btc, x_eth, x_sol)]
    # fold v-bias and o-bias: out = attn @ Wo + (sum_j bv_j) @ Wo + bo
    bo2 = (np.asarray(bo, np.float64)
           + np.asarray(bv, np.float64).sum(0) @ np.asarray(Wo, np.float64))
    common = {
        "Wq": np.asarray(Wq, np.float32).astype(bff),
        "bq": np.asarray(bq, np.float32),
        "Wk": np.asarray(Wk, np.float32).astype(bff),
        "Wv": np.asarray(Wv, np.float32).astype(bff),
        "Wo": np.asarray(Wo, np.float32).astype(bff),
        "bo2": bo2.astype(np.float32).astype(bff),
    }
    in_maps = []
    for c in range(N_CORES):
        b, sq = c // 4, c % 4
        # Roll the sequence so this core's query quarter sits at rows [0:512)
        # (the kernel always projects q from rows 0:512).  k/v see the rolled
        # full sequence, which is fine: softmax+sum over the key axis is
        # permutation-invariant.
        xq = np.stack(
            [np.roll(xs[i][b], -sq * SQ, axis=0) for i in range(3)]
        ).astype(bff)
        in_maps.append({"x": np.ascontiguousarray(xq), **common})
    import os
    res = run_bass_kernel_spmd(
        nc, in_maps, core_ids=list(range(N_CORES)),
        trace=bool(os.environ.get("BASS_TRACE")),
    )
    _CACHE["last_res"] = res

    outs = [np.empty((B, S, D), np.float32) for _ in range(3)]
    for c in range(N_CORES):
        b, sq = c // 4, c % 4
        o = res.results[c]["out"]
        for i in range(3):
            outs[i][b, sq * SQ : (sq + 1) * SQ] = o[i]
    return tuple(outs)


if __name__ == "__main__":
    import reference

    inp = reference.setup_inputs()
    inp = {k: np.asarray(v) for k, v in inp.items()}
    got = kernel(**inp)
    exp = reference.reference(**inp)
    for i in range(3):
        g, e = np.asarray(got[i]), np.asarray(exp[i])
        err = np.abs(g - e).max() / np.abs(e).max()
        print(f"out[{i}] rel err {err:.3e}")


# revision 9
# speedup vs baseline: 1.3289x; 1.0329x over previous
"""Cross-cryptocurrency attention kernel for 8 Trainium2 NeuronCores.

Sharding: 16 (batch, seq-quarter) shards -> core c handles b = c//4,
query rows s in [512*(c%4), 512*(c%4+1)).  Each core computes all 8 heads
and all 9 (query-asset, key-asset) pairs for its query slice, with full
keys/values (S=2048) for its batch, so the output projection is local and
no collectives are needed.

v2 design (ACT exp is the hard floor; everything else tucks under it):
  - host folds biases: bk drops out of softmax exactly; bv folds into
    bo2 = bo + (sum_j bv_j) @ Wo; x/weights pre-cast to bf16.
  - projections all-bf16 (1 PE cycle/row instead of 4 for f32).
  - scores^T[t,s] on PE (lhsT=k^T bf16), exp on ACT (PSUM->SBUF bf16).
  - AV with the *E tile* as the stationary operand: out[s,33] accumulates
    over 16 t-tiles at 33 rows each (vs 512) -- 4x less PE time; the
    ones column of [v|1] yields row-sums Z in col 32.
  - normalize straight out of the AV psum: DVE reciprocal of Z +
    scalar_tensor_tensor (mul rr, add acc) -- no PE transpose needed.
  - software pipeline: per score group g: PE scores(g) -> ACT exp(g) ->
    PE AV(g-1); asset-1/2 projections drip between early combos so ACT
    never starves.
"""

import math
import numpy as np

B = 2
S = 2048
D = 256
H = 8
HD = 32
SQ = 512  # query rows per core
N_CORES = 8
SCALE = 1.0 / math.sqrt(HD)
# 2-tile groups at both ends: the boundary exps are 1024 elems, long enough
# to cover the next combo's first score group catching up through the
# 2-deep psum ring (a trailing 512-elem exp leaves a ~300ns ACT gap there)
GROUPS = [(0, 2), (2, 3), (5, 3), (8, 3), (11, 3), (14, 2)]

_CACHE = {}


def _build():
    from contextlib import ExitStack

    import concourse.bass as bass
    import concourse.mybir as mybir
    import concourse.tile as tile
    from concourse import bacc
    from concourse.masks import make_identity

    f32 = mybir.dt.float32
    bf16 = mybir.dt.bfloat16
    AF = mybir.ActivationFunctionType
    ALU = mybir.AluOpType

    nc = bacc.Bacc("TRN2", target_bir_lowering=False, debug=False)

    x_d = nc.dram_tensor("x", [3, S, D], bf16, kind="ExternalInput").ap()
    Wq_d = nc.dram_tensor("Wq", [3, D, D], bf16, kind="ExternalInput").ap()
    bq_d = nc.dram_tensor("bq", [3, D], f32, kind="ExternalInput").ap()
    Wk_d = nc.dram_tensor("Wk", [3, D, D], bf16, kind="ExternalInput").ap()
    Wv_d = nc.dram_tensor("Wv", [3, D, D], bf16, kind="ExternalInput").ap()
    Wo_d = nc.dram_tensor("Wo", [D, D], bf16, kind="ExternalInput").ap()
    bo2_d = nc.dram_tensor("bo2", [D], bf16, kind="ExternalInput").ap()
    out_d = nc.dram_tensor("out", [3, SQ, D], f32, kind="ExternalOutput").ap()

    with tile.TileContext(nc) as tc, ExitStack() as ctx:
        const_p = ctx.enter_context(tc.tile_pool(name="const", bufs=1))
        xT_p = ctx.enter_context(tc.tile_pool(name="xT", bufs=1))
        qkv_p = ctx.enter_context(tc.tile_pool(name="qkv", bufs=1))
        acc_p = ctx.enter_context(tc.tile_pool(name="acc", bufs=1))
        xn_p = ctx.enter_context(tc.tile_pool(name="xn", bufs=2))
        e_p = ctx.enter_context(tc.tile_pool(name="epool", bufs=4))
        sm_p = ctx.enter_context(tc.tile_pool(name="small", bufs=2))
        # PSUM: 3+3 (score groups) + 1+1 (AV accum) = 8 banks
        ps_S = ctx.enter_context(tc.tile_pool(name="psS", bufs=2, space="PSUM"))
        ps_A = ctx.enter_context(tc.tile_pool(name="psA", bufs=2, space="PSUM"))

        # ---- constants / weights to SBUF ----
        ident = const_p.tile([128, 128], f32)
        make_identity(nc, ident[:])
        identb = const_p.tile([128, 128], bf16)
        make_identity(nc, identb[:])
        onesb = const_p.tile([1, 128], bf16)
        nc.gpsimd.memset(onesb[:], 1.0)

        wq_sb = const_p.tile([128, 3 * 2 * D], bf16)
        wk_sb = const_p.tile([128, 3 * 2 * D], bf16)
        wv_sb = const_p.tile([128, 3 * 2 * D], bf16)
        for w_sb, w_d in ((wq_sb, Wq_d), (wk_sb, Wk_d), (wv_sb, Wv_d)):
            nc.sync.dma_start(
                w_sb[:].rearrange("p (a kt f) -> p a kt f", a=3, kt=2),
                w_d.rearrange("a (kt p) f -> p a kt f", p=128),
            )
        wo_sb = const_p.tile([128, 2 * D], bf16)
        nc.sync.dma_start(
            wo_sb[:].rearrange("p (kt f) -> p kt f", kt=2),
            Wo_d.rearrange("(kt p) f -> p kt f", p=128),
        )
        bq_sb = const_p.tile([128, 6], f32)  # col = a*2 + dt
        nc.sync.dma_start(
            bq_sb[:].rearrange("p (a dt) -> p a dt", a=3),
            bq_d.rearrange("a (dt p) -> p a dt", p=128),
        )
        bo2_row = const_p.tile([1, D], bf16)
        nc.sync.dma_start(bo2_row[:], bo2_d[None, :])

        # ---- per-asset persistent tensors ----
        xT = [xT_p.tile([128, 2 * S], bf16, tag=f"xT{_}", name=f"xT{_}") for _ in range(3)]
        kT = [qkv_p.tile([128, 2 * S], bf16, tag=f"kT{_}", name=f"kT{_}") for _ in range(3)]
        qT = [qkv_p.tile([128, 2 * SQ], bf16, tag=f"qT{_}", name=f"qT{_}") for _ in range(3)]
        v1 = [qkv_p.tile([128, 16 * (H * 33)], bf16, tag=f"v1_{_}", name=f"v1_{_}") for _ in range(3)]
        out_acc = [acc_p.tile([128, 4 * D], f32, tag=f"oacc{_}", name=f"oacc{_}") for _ in range(3)]

        # ======== Phase 1 as a unit generator (dripped between combos) ====
        def proj_units(a):
            xn = xn_p.tile([128, 16 * D], bf16, tag="xn", name=f"xn{a}")

            def dma_unit(c):
                def run():
                    nc.sync.dma_start(
                        xn[:, c * 4 * D : (c + 1) * 4 * D].rearrange(
                            "p (st d) -> p st d", st=4
                        ),
                        x_d[a].rearrange("(st p) d -> p st d", p=128)[:, 4 * c : 4 * c + 4],
                    )
                return run

            for c in range(4):
                yield dma_unit(c)

            def transp_unit(dt, g):
                def run():
                    pst = ps_S.tile([128, 512], bf16, tag="psS", name="ps1")
                    for u in range(4):
                        st = 4 * g + u
                        nc.tensor.transpose(
                            pst[:, u * 128 : (u + 1) * 128],
                            xn[:, st * D + dt * 128 : st * D + dt * 128 + 128],
                            identb[:],
                        )
                    nc.vector.tensor_copy(
                        xT[a][:, dt * S + g * 512 : dt * S + (g + 1) * 512], pst[:]
                    )
                return run

            for dt in range(2):
                for g in range(4):
                    yield transp_unit(dt, g)

            # k^T projection (bias bk dropped: softmax shift-invariant)
            def k_unit(dt, tc4):
                def run():
                    psk = ps_S.tile([128, 512], f32, tag="psS", name="ps1")
                    for kt in range(2):
                        nc.tensor.matmul(
                            psk[:],
                            wk_sb[:, a * 2 * D + kt * D + dt * 128 : a * 2 * D + kt * D + dt * 128 + 128],
                            xT[a][:, kt * S + tc4 * 512 : kt * S + (tc4 + 1) * 512],
                            start=(kt == 0),
                            stop=(kt == 1),
                        )
                    nc.vector.tensor_copy(
                        kT[a][:, dt * S + tc4 * 512 : dt * S + (tc4 + 1) * 512], psk[:]
                    )
                return run

            # q^T projection (+ bq)
            def q_unit(dt):
                def run():
                    psq = ps_S.tile([128, 512], f32, tag="psS", name="ps1")
                    for kt in range(2):
                        nc.tensor.matmul(
                            psq[:],
                            wq_sb[:, a * 2 * D + kt * D + dt * 128 : a * 2 * D + kt * D + dt * 128 + 128],
                            xT[a][:, kt * S : kt * S + SQ],
                            start=(kt == 0),
                            stop=(kt == 1),
                        )
                    nc.vector.tensor_scalar_add(
                        qT[a][:, dt * SQ : (dt + 1) * SQ],
                        psq[:],
                        bq_sb[:, a * 2 + dt : 1 + a * 2 + dt],
                    )
                return run

            for dt in range(2):
                yield q_unit(dt)
            for dt in range(2):
                for tc4 in range(4):
                    yield k_unit(dt, tc4)

            # v projection (bias bv folded into bo2 host-side)
            def ones_unit():
                def run():
                    nc.gpsimd.memset(
                        v1[a].rearrange("p (t h x) -> p (t h) x", h=H, x=33)[:, :, 32:33],
                        1.0,
                    )
                return run

            yield ones_unit()

            # v projection: two t-tiles per unit sharing one psum bank (the
            # second chain rides the first's lazy-zeroed region)
            def v_unit(pr):
                def run():
                    psv = ps_S.tile([128, 512], f32, tag="psS", name="psv")
                    for half in range(2):
                        st = 2 * pr + half
                        for kt in range(2):
                            nc.tensor.matmul(
                                psv[:, half * D : (half + 1) * D],
                                xT[a][:, kt * S + st * 128 : kt * S + (st + 1) * 128],
                                wv_sb[:, a * 2 * D + kt * D : a * 2 * D + (kt + 1) * D],
                                start=(half == 0 and kt == 0),
                                stop=(half == 1 and kt == 1),
                                skip_group_check=True,
                            )
                    dst = v1[a][
                        :, 2 * pr * (H * 33) : (2 * pr + 2) * (H * 33)
                    ].rearrange("p (s2 h x) -> p s2 h x", s2=2, x=33)[:, :, :, 0:32]
                    nc.vector.tensor_copy(
                        dst, psv[:].rearrange("p (s2 h x) -> p s2 h x", s2=2, x=32)
                    )
                return run

            for pr in range(8):
                yield v_unit(pr)

        # ======== Phase 2: one (i, j, h) combo ========
        def emit_av(eg, t0, glen, j, h, psA):
            # One accumulation group for the whole bank: start=True lazily
            # zeroes the full 2KB zero region, so only the very first matmul
            # may set it; the other three s-subtile chains accumulate onto
            # pending-zero bytes.
            for u in range(glen):
                tt = t0 + u
                for k in range(4):
                    nc.tensor.matmul(
                        psA[:, k * 33 : (k + 1) * 33],
                        eg[:, u * 512 + k * 128 : u * 512 + (k + 1) * 128],
                        v1[j][:, tt * (H * 33) + h * 33 : tt * (H * 33) + (h + 1) * 33],
                        start=(tt == 0 and k == 0),
                        stop=(tt == 15 and k == 3),
                        skip_group_check=True,
                    )

        def norm_unit(i, j, h, psA):
            # normalize from AV psum: rr = 1/Z, acc += O * rr
            def run():
                rr4 = sm_p.tile([128, 4], f32, tag="rr", name="rr")
                nc.vector.reciprocal_approx_fast(
                    rr4[:],
                    psA[:].rearrange("p (k x) -> p k x", x=33)[:, :, 32],
                )
                for k in range(4):
                    oa = out_acc[i][:, k * D + h * 32 : k * D + (h + 1) * 32]
                    src = psA[:, k * 33 : k * 33 + 32]
                    if j == 0:
                        nc.vector.tensor_scalar_mul(oa, src, rr4[:, k : k + 1])
                    else:
                        nc.vector.scalar_tensor_tensor(
                            oa, src, rr4[:, k : k + 1], oa, op0=ALU.mult, op1=ALU.add
                        )
            return run

        def combo(i, j, h, tail, drip):
            """Emit one (i,j,h) combo.  `tail` is the deferred work of the
            previous combo (its last two AV batches + normalize), flushed
            right after this combo's first score group so the PE has already
            queued scores(g0) when the previous combo's last exp retires.
            Returns this combo's own tail.  `drip(n)` emits up to n pairs of
            background psum units at parity-neutral points."""
            hp = 32 * (h % 4)
            hc = h // 4
            psA = ps_A.tile([128, 4 * 33], f32, tag="psA", name="psA")
            egs = []

            def sc(gi):
                t0, glen = GROUPS[gi]
                psS = ps_S.tile([128, glen * 512], f32, tag="psS", name="ps2")
                for u in range(glen):
                    tt = t0 + u
                    nc.tensor.matmul(
                        psS[:, u * 512 : (u + 1) * 512],
                        kT[j][hp : hp + 32, hc * S + tt * 128 : hc * S + (tt + 1) * 128],
                        qT[i][hp : hp + 32, hc * SQ : (hc + 1) * SQ],
                        start=True,
                        stop=True,
                        tile_position=(hp, 0),
                    )
                eg = e_p.tile([128, 3 * 512], bf16, tag="eg", name="eg")
                nc.scalar.activation(eg[:, 0 : glen * 512], psS[:], AF.Exp, scale=SCALE)
                egs.append((eg, t0, glen))

            sc(0)
            for t in tail:
                t()
            sc(1)
            emit_av(*egs[0], j, h, psA)
            sc(2)
            drip(2)
            emit_av(*egs[1], j, h, psA)
            sc(3)
            emit_av(*egs[2], j, h, psA)
            sc(4)
            emit_av(*egs[3], j, h, psA)
            sc(5)
            return [
                lambda: emit_av(*egs[4], j, h, psA),
                lambda: emit_av(*egs[5], j, h, psA),
                norm_unit(i, j, h, psA),
            ]

        # ======== Phase 3: output projection for one asset, as units ======
        def phase3_units(i):
            aT = acc_p.tile([128, 2 * SQ], bf16, tag=f"aT{i}", name=f"aT{i}")

            def t_unit(dt):
                def run():
                    pst = ps_S.tile([128, 512], f32, tag="psS", name="ps3")
                    for st in range(4):
                        nc.tensor.transpose(
                            pst[:, st * 128 : (st + 1) * 128],
                            out_acc[i][:, st * D + dt * 128 : st * D + dt * 128 + 128],
                            ident[:],
                        )
                    nc.vector.tensor_copy(aT[:, dt * SQ : (dt + 1) * SQ], pst[:])
                return run

            for dt in range(2):
                yield t_unit(dt)

            def p_unit(st):
                def run():
                    psf = ps_S.tile([128, D], f32, tag="psS", name="psf")
                    for dt in range(2):
                        nc.tensor.matmul(
                            psf[:],
                            aT[:, dt * SQ + st * 128 : dt * SQ + (st + 1) * 128],
                            wo_sb[:, dt * D : (dt + 1) * D],
                            start=(dt == 0),
                            stop=False,
                        )
                    nc.tensor.matmul(
                        psf[:],
                        onesb[0:1, 0:128],
                        bo2_row[0:1, :],
                        start=False,
                        stop=True,
                    )
                    ot = sm_p.tile([128, D], f32, tag="ot", name="ot")
                    nc.vector.tensor_copy(ot[:], psf[:])
                    nc.sync.dma_start(
                        out_d[i].rearrange("(st p) d -> st p d", p=128)[st], ot[:]
                    )
                return run

            for st in range(4):
                yield p_unit(st)

        # ======== Emission schedule ========
        for u in proj_units(0):
            u()

        # Deadline-tagged drip queue: (unit, not_before_combo).  Assets 1/2
        # projections must land before their first combos ((0,1,*) at 8,
        # (0,2,*) at 16 -- both met with huge slack at 4 units/combo);
        # phase3(i) units wait for asset i's last normalize (flushed at the
        # start of combo 24*(i+1)).
        dripq = []
        for a in (1, 2):
            for u in proj_units(a):
                dripq.append((u, 0))
        for idx, u in enumerate(phase3_units(0)):
            dripq.append((u, 24 + idx // 2))
        for idx, u in enumerate(phase3_units(1)):
            dripq.append((u, 48 + idx // 2))

        combo_idx = [0]

        def drip(n_pairs):
            budget = 2 * n_pairs
            while dripq and budget > 0 and dripq[0][1] <= combo_idx[0]:
                u, _ = dripq.pop(0)
                u()
                budget -= 1

        tail = []
        for i in range(3):
            for j in range(3):
                for h in range(H):
                    tail = combo(i, j, h, tail, drip)
                    combo_idx[0] += 1
        for t in tail:
            t()
        while dripq:
            dripq.pop(0)[0]()
        for u in phase3_units(2):
            u()
    nc.compile()
    return nc


def kernel(x_btc, x_eth, x_sol, Wq, bq, Wk, bk, Wv, bv, Wo, bo):
    import ml_dtypes
    from concourse.bass_utils import run_bass_kernel_spmd

    if "nc" not in _CACHE:
        _CACHE["nc"] = _build()
    nc = _CACHE["nc"]

    bff = ml_dtypes.bfloat16
    xs = [np.asarray(t, dtype=np.float32) for t in (x_btc, x_eth, x_sol)]
    # fold v-bias and o-bias: out = attn @ Wo + (sum_j bv_j) @ Wo + bo
    bo2 = (np.asarray(bo, np.float64)
           + np.asarray(bv, np.float64).sum(0) @ np.asarray(Wo, np.float64))
    common = {
        "Wq": np.asarray(Wq, np.float32).astype(bff),
        "bq": np.asarray(bq, np.float32),
        "Wk": np.asarray(Wk, np.float32).astype(bff),
        "Wv": np.asarray(Wv, np.float32).astype(bff),
        "Wo": np.asarray(Wo, np.float32).astype(bff),
        "bo2": bo2.astype(np.float32).astype(bff),
    }
    in_maps = []
    for c in range(N_CORES):
        b, sq = c // 4, c % 4
        # Roll the sequence so this core's query quarter sits at rows [0:512)
        # (the kernel always projects q from rows 0:512).  k/v see the rolled
        # full sequence, which is fine: softmax+sum over the key axis is
        # permutation-invariant.
        xq = np.stack(
            [np.roll(xs[i][b], -sq * SQ, axis=0) for i in range(3)]
        ).astype(bff)
        in_maps.append({"x": np.ascontiguousarray(xq), **common})
    import os
    res = run_bass_kernel_spmd(
        nc, in_maps, core_ids=list(range(N_CORES)),
        trace=bool(os.environ.get("BASS_TRACE")),
    )
    _CACHE["last_res"] = res

    outs = [np.empty((B, S, D), np.float32) for _ in range(3)]
    for c in range(N_CORES):
        b, sq = c // 4, c % 4
        o = res.results[c]["out"]
        for i in range(3):
            outs[i][b, sq * SQ : (sq + 1) * SQ] = o[i]
    return tuple(outs)


if __name__ == "__main__":
    import reference

    inp = reference.setup_inputs()
    inp = {k: np.asarray(v) for k, v in inp.items()}
    got = kernel(**inp)
    exp = reference.reference(**inp)
    for i in range(3):
        g, e = np.asarray(got[i]), np.asarray(exp[i])
        err = np.abs(g - e).max() / np.abs(e).max()
        print(f"out[{i}] rel err {err:.3e}")


# revision 10
# speedup vs baseline: 1.3395x; 1.0080x over previous
"""Cross-cryptocurrency attention kernel for 8 Trainium2 NeuronCores.

Sharding: 16 (batch, seq-quarter) shards -> core c handles b = c//4,
query rows s in [512*(c%4), 512*(c%4+1)).  Each core computes all 8 heads
and all 9 (query-asset, key-asset) pairs for its query slice, with full
keys/values (S=2048) for its batch, so the output projection is local and
no collectives are needed.

v3 design.  ACT exp (75.5M exps/core -> ~572us incl per-inst overhead) is
the hard floor; everything else is organised to hide under it:
  - host folds biases (bk drops out of softmax exactly; bv/bo fold into
    bo2 = bo + (sum_j bv_j) @ Wo) and pre-packs x/weights in bf16 in the
    exact SBUF layouts, so startup DMAs are few and contiguous.
  - x is transposed by the DMA xbar (dma_start_transpose), zero PE cost.
  - all projections run as [128,256]-output chunk units through a
    dedicated psum bank (psD halves, DVE-memset + start=False chains),
    fully decoupled from the score-psum ring so background work never
    stalls the PE->ACT score pipeline.
  - scores^T[t,s] on PE (lhsT=k^T bf16) in groups (2,3,3,3,3,2); exp on
    ACT PSUM->SBUF bf16.  2-tile groups at the combo boundaries cover the
    next combo's catch-up through the 2-deep score ring.
  - AV with the E tile stationary: out[s,33] accumulates over 16 t-tiles
    at 33 rows each (4x less PE than v-stationary); the ones column of
    [v|1] yields row-sums Z in col 32.  Both combo accumulators pack into
    one psum bank (halves, DVE-memset + start=False).
  - normalize straight off the AV psum: DVE reciprocal + STT mul-add.
  - software pipeline: per combo, the last two AV batches + normalize
    defer into the next combo right after its first score group, so ACT
    never waits at combo boundaries; projection/phase-3 units drip at
    four points per combo with deadlines.
"""

import math
import numpy as np

B = 2
S = 2048
D = 256
H = 8
HD = 32
SQ = 512  # query rows per core
N_CORES = 8
SCALE = 1.0 / math.sqrt(HD)
# 2-tile groups at both ends: boundary exps are 1024 elems, long enough to
# cover the next combo's first score group catching up through the ring.
GROUPS = [(0, 2), (2, 3), (5, 3), (8, 3), (11, 3), (14, 2)]

_CACHE = {}


def _build():
    from contextlib import ExitStack

    import concourse.bass as bass
    import concourse.mybir as mybir
    import concourse.tile as tile
    from concourse import bacc
    from concourse.masks import make_identity

    f32 = mybir.dt.float32
    bf16 = mybir.dt.bfloat16
    AF = mybir.ActivationFunctionType
    ALU = mybir.AluOpType

    nc = bacc.Bacc("TRN2", target_bir_lowering=False, debug=False)

    x_d = nc.dram_tensor("x", [3, S, D], bf16, kind="ExternalInput").ap()
    # host-packed weights: [p, (a, ty q/k/v, kt, f)] bf16 and [p, (a,dt)] f32
    wpack_d = nc.dram_tensor("wpack", [128, 3 * 3 * 2 * D], bf16, kind="ExternalInput").ap()
    wo_d = nc.dram_tensor("wo", [128, 2 * D], bf16, kind="ExternalInput").ap()
    bq_d = nc.dram_tensor("bqp", [128, 6], f32, kind="ExternalInput").ap()
    bo2_d = nc.dram_tensor("bo2", [D], bf16, kind="ExternalInput").ap()
    out_d = nc.dram_tensor("out", [3, SQ, D], f32, kind="ExternalOutput").ap()

    with tile.TileContext(nc) as tc, ExitStack() as ctx:
        const_p = ctx.enter_context(tc.tile_pool(name="const", bufs=1))
        qkv_p = ctx.enter_context(tc.tile_pool(name="qkv", bufs=1))
        acc_p = ctx.enter_context(tc.tile_pool(name="acc", bufs=1))
        e_p = ctx.enter_context(tc.tile_pool(name="epool", bufs=4))
        sm_p = ctx.enter_context(tc.tile_pool(name="small", bufs=2))
        # PSUM: 3+3 score ring + 1 packed AV accumulators + 1 drip bank
        ps_S = ctx.enter_context(tc.tile_pool(name="psS", bufs=2, space="PSUM"))
        psAB_p = ctx.enter_context(tc.tile_pool(name="psAB", bufs=1, space="PSUM"))
        psD_p = ctx.enter_context(tc.tile_pool(name="psD", bufs=1, space="PSUM"))
        psAB = psAB_p.tile([128, 512], f32, name="psAB")
        psD = psD_p.tile([128, 512], f32, name="psD")

        # ---- constants / weights to SBUF (bq + asset-0 weights first so
        # the first score group can start ~3us in) ----
        bq_sb = const_p.tile([128, 6], f32)
        nc.sync.dma_start(bq_sb[:], bq_d)
        wsb = const_p.tile([128, 3 * 3 * 2 * D], bf16)
        nc.sync.dma_start(wsb[:, 0 : 3 * 2 * D], wpack_d[:, 0 : 3 * 2 * D])

        xT = [qkv_p.tile([128, 2 * S], bf16, tag=f"xT{_}", name=f"xT{_}") for _ in range(3)]
        kT = [qkv_p.tile([128, 2 * S], bf16, tag=f"kT{_}", name=f"kT{_}") for _ in range(3)]
        qT = [qkv_p.tile([128, 2 * SQ], bf16, tag=f"qT{_}", name=f"qT{_}") for _ in range(3)]
        v1 = [qkv_p.tile([128, 16 * (H * 33)], bf16, tag=f"v1_{_}", name=f"v1_{_}") for _ in range(3)]
        out_acc = [acc_p.tile([128, 4 * D], f32, tag=f"oacc{_}", name=f"oacc{_}") for _ in range(3)]

        def dmaT_unit(a, c):
            def run():
                nc.sync.dma_start_transpose(
                    xT[a].rearrange("p (dt s) -> p dt s", dt=2)[:, :, c * 512 : (c + 1) * 512],
                    x_d[a][c * 512 : (c + 1) * 512, :],
                )
            return run

        # transpose chunks for asset 0 + the rest of the constants
        for c in range(4):
            dmaT_unit(0, c)()
        nc.sync.dma_start(wsb[:, 3 * 2 * D :], wpack_d[:, 3 * 2 * D :])
        wo_sb = const_p.tile([128, 2 * D], bf16)
        nc.sync.dma_start(wo_sb[:], wo_d)
        bo2_row = const_p.tile([1, D], bf16)
        nc.sync.dma_start(bo2_row[:], bo2_d[None, :])
        ident = const_p.tile([128, 128], f32)
        make_identity(nc, ident[:])
        onesb = const_p.tile([1, 128], bf16)
        nc.gpsimd.memset(onesb[:], 1.0)

        # ---- drip bank: [128,256] halves, DVE-memset + start=False ----
        dctr = [0]

        def dhalf():
            hh = dctr[0] & 1
            dctr[0] += 1
            reg = psD[:, hh * 256 : (hh + 1) * 256]
            nc.vector.memset(reg, 0.0)
            return reg

        WT_Q, WT_K, WT_V = 0, 1, 2

        def wcol(a, ty, kt, off, width):
            base = a * (3 * 2 * D) + ty * (2 * D) + kt * D + off
            return wsb[:, base : base + width]

        def k_unit(a, dt, n):  # kT[a] cols [dt*S + 256n, +256)
            def run():
                reg = dhalf()
                for kt in range(2):
                    nc.tensor.matmul(
                        reg,
                        wcol(a, WT_K, kt, dt * 128, 128),
                        xT[a][:, kt * S + n * 256 : kt * S + (n + 1) * 256],
                        start=False, stop=(kt == 1), skip_group_check=True,
                    )
                nc.vector.tensor_copy(
                    kT[a][:, dt * S + n * 256 : dt * S + (n + 1) * 256], reg
                )
            return run

        def q_unit(a, dt, n):  # qT[a] cols [dt*SQ + 256n, +256)
            def run():
                reg = dhalf()
                for kt in range(2):
                    nc.tensor.matmul(
                        reg,
                        wcol(a, WT_Q, kt, dt * 128, 128),
                        xT[a][:, kt * S + n * 256 : kt * S + (n + 1) * 256],
                        start=False, stop=(kt == 1), skip_group_check=True,
                    )
                nc.vector.tensor_scalar_add(
                    qT[a][:, dt * SQ + n * 256 : dt * SQ + (n + 1) * 256],
                    reg,
                    bq_sb[:, a * 2 + dt : a * 2 + dt + 1],
                )
            return run

        def v_unit(a, st):
            def run():
                reg = dhalf()
                for kt in range(2):
                    nc.tensor.matmul(
                        reg,
                        xT[a][:, kt * S + st * 128 : kt * S + (st + 1) * 128],
                        wcol(a, WT_V, kt, 0, D),
                        start=False, stop=(kt == 1), skip_group_check=True,
                    )
                dst = v1[a][
                    :, st * (H * 33) : (st + 1) * (H * 33)
                ].rearrange("p (h x) -> p h x", x=33)[:, :, 0:32]
                nc.vector.tensor_copy(dst, reg.rearrange("p (h x) -> p h x", x=32))
            return run

        def ones_unit(a):
            def run():
                nc.gpsimd.memset(
                    v1[a].rearrange("p (t h x) -> p (t h) x", h=H, x=33)[:, :, 32:33],
                    1.0,
                )
            return run

        # ======== Phase 2: one (i, j, h) combo ========
        def emit_av(eg, t0, glen, j, h, reg):
            for u in range(glen):
                tt = t0 + u
                for k in range(4):
                    nc.tensor.matmul(
                        reg[:, k * 33 : (k + 1) * 33],
                        eg[:, u * 512 + k * 128 : u * 512 + (k + 1) * 128],
                        v1[j][:, tt * (H * 33) + h * 33 : tt * (H * 33) + (h + 1) * 33],
                        start=False,
                        stop=(tt == 15 and k == 3),
                        skip_group_check=True,
                    )

        def norm_unit(i, j, h, reg):
            def run():
                rr4 = sm_p.tile([128, 4], f32, tag="rr", name="rr")
                nc.vector.reciprocal_approx_fast(
                    rr4[:],
                    reg.rearrange("p (k x) -> p k x", x=33)[:, :, 32],
                )
                for k in range(4):
                    oa = out_acc[i][:, k * D + h * 32 : k * D + (h + 1) * 32]
                    src = reg[:, k * 33 : k * 33 + 32]
                    if j == 0:
                        nc.vector.tensor_scalar_mul(oa, src, rr4[:, k : k + 1])
                    else:
                        nc.vector.scalar_tensor_tensor(
                            oa, src, rr4[:, k : k + 1], oa, op0=ALU.mult, op1=ALU.add
                        )
            return run

        def combo(ci, i, j, h, tail, drip, pre_sc=None, pre_av=None, pre_tail=None):
            hp = 32 * (h % 4)
            hc = h // 4
            reg = psAB[:, (ci % 2) * 256 : (ci % 2) * 256 + 132]
            nc.vector.memset(reg, 0.0)
            egs = []

            def sc(gi):
                t0, glen = GROUPS[gi]
                psS = ps_S.tile([128, glen * 512], f32, tag="psS", name="ps2")
                for u in range(glen):
                    tt = t0 + u
                    nc.tensor.matmul(
                        psS[:, u * 512 : (u + 1) * 512],
                        kT[j][hp : hp + 32, hc * S + tt * 128 : hc * S + (tt + 1) * 128],
                        qT[i][hp : hp + 32, hc * SQ : (hc + 1) * SQ],
                        start=True,
                        stop=True,
                        tile_position=(hp, 0),
                    )
                eg = e_p.tile([128, 3 * 512], bf16, tag="eg", name="eg")
                nc.scalar.activation(eg[:, 0 : glen * 512], psS[:], AF.Exp, scale=SCALE)
                egs.append((eg, t0, glen))

            def hook(d, gi):
                if d and gi in d:
                    for u in d[gi]:
                        u()

            hook(pre_sc, 0)
            sc(0)
            if pre_tail:
                for u in pre_tail:
                    u()
            for t in tail:
                t()
            hook(pre_sc, 1)
            sc(1)
            hook(pre_av, 0)
            drip(2)
            emit_av(*egs[0], j, h, reg)
            hook(pre_sc, 2)
            sc(2)
            hook(pre_av, 1)
            drip(2)
            emit_av(*egs[1], j, h, reg)
            hook(pre_sc, 3)
            sc(3)
            hook(pre_av, 2)
            drip(2)
            emit_av(*egs[2], j, h, reg)
            hook(pre_sc, 4)
            sc(4)
            hook(pre_av, 3)
            drip(2)
            emit_av(*egs[3], j, h, reg)
            hook(pre_sc, 5)
            sc(5)
            return [
                lambda: emit_av(*egs[4], j, h, reg),
                lambda: emit_av(*egs[5], j, h, reg),
                norm_unit(i, j, h, reg),
            ]

        # ======== Phase 3: output projection for one asset, as units ======
        aT = [acc_p.tile([128, 2 * SQ], bf16, tag=f"aT{_}", name=f"aT{_}") for _ in range(3)]

        def t_unit(i, dt, half):  # transpose out_acc block -> aT bf16
            def run():
                reg = dhalf()
                for k in range(2):
                    st = 2 * half + k
                    nc.tensor.matmul(
                        reg[:, k * 128 : (k + 1) * 128],
                        out_acc[i][:, st * D + dt * 128 : st * D + dt * 128 + 128],
                        ident[:],
                        is_transpose=True,
                        start=False, stop=True, skip_group_check=True,
                    )
                nc.vector.tensor_copy(
                    aT[i][:, dt * SQ + half * 256 : dt * SQ + (half + 1) * 256], reg
                )
            return run

        def p_unit(i, st):
            def run():
                reg = dhalf()
                for dt in range(2):
                    nc.tensor.matmul(
                        reg,
                        aT[i][:, dt * SQ + st * 128 : dt * SQ + (st + 1) * 128],
                        wo_sb[:, dt * D : (dt + 1) * D],
                        start=False, stop=False, skip_group_check=True,
                    )
                nc.tensor.matmul(
                    reg,
                    onesb[0:1, 0:128],
                    bo2_row[0:1, :],
                    start=False, stop=True, skip_group_check=True,
                )
                ot = sm_p.tile([128, D], f32, tag="ot", name="ot")
                nc.vector.tensor_copy(ot[:], reg)
                nc.sync.dma_start(
                    out_d[i].rearrange("(st p) d -> st p d", p=128)[st], ot[:]
                )
            return run

        def ph3_units(i):
            return [t_unit(i, dt, half) for dt in range(2) for half in range(2)] + [
                p_unit(i, st) for st in range(4)
            ]

        # ======== Emission schedule ========
        # startup prefix: just enough for combo (0,0,0) group 0
        q_unit(0, 0, 0)()
        q_unit(0, 0, 1)()
        k_unit(0, 0, 0)()
        ones_unit(0)()

        # combo-0/1 custom placement: asset-0 dt0 k-units and v-units land
        # exactly where the score groups / AV batches need them
        c0_pre_sc = {
            1: [k_unit(0, 0, 1), k_unit(0, 0, 2)],
            2: [k_unit(0, 0, 3)],
            3: [k_unit(0, 0, 4), k_unit(0, 0, 5)],
            4: [k_unit(0, 0, 6)],
            5: [k_unit(0, 0, 7)],
        }
        c0_pre_av = {
            0: [v_unit(0, 0), v_unit(0, 1)],
            1: [v_unit(0, 2), v_unit(0, 3), v_unit(0, 4)],
            2: [v_unit(0, 5), v_unit(0, 6), v_unit(0, 7)],
            3: [v_unit(0, 8), v_unit(0, 9), v_unit(0, 10)],
        }
        c1_pre_tail = [v_unit(0, st) for st in range(11, 16)]

        # deadline-tagged drip queue
        dripq = []
        for dt in range(2):
            for n in range(2):
                dripq.append((q_unit(0, 1, n), 0))
        for n in range(8):
            dripq.append((k_unit(0, 1, n), 0))
        for a in (1, 2):
            for c in range(4):
                dripq.append((dmaT_unit(a, c), 0))
            dripq.append((ones_unit(a), 0))
            for dt in range(2):
                for n in range(2):
                    dripq.append((q_unit(a, dt, n), 0))
            for dt in range(2):
                for n in range(8):
                    dripq.append((k_unit(a, dt, n), 0))
            for st in range(16):
                dripq.append((v_unit(a, st), 0))
        for u in ph3_units(0):
            dripq.append((u, 24))
        for u in ph3_units(1):
            dripq.append((u, 48))

        ci_box = [0]

        def drip(budget):
            while dripq and budget > 0 and dripq[0][1] <= ci_box[0]:
                dripq.pop(0)[0]()
                budget -= 1

        tail = []
        ci = 0
        for i in range(3):
            for j in range(3):
                for h in range(H):
                    ci_box[0] = ci
                    combo_kw = {}
                    if ci == 0:
                        combo_kw = dict(pre_sc=c0_pre_sc, pre_av=c0_pre_av)
                    elif ci == 1:
                        combo_kw = dict(pre_tail=c1_pre_tail)
                    tail = combo(ci, i, j, h, tail, drip, **combo_kw)
                    ci += 1
        for t in tail:
            t()
        while dripq:
            dripq.pop(0)[0]()
        for u in ph3_units(2):
            u()
    nc.compile()
    return nc


def kernel(x_btc, x_eth, x_sol, Wq, bq, Wk, bk, Wv, bv, Wo, bo):
    import ml_dtypes
    from concourse.bass_utils import run_bass_kernel_spmd

    if "nc" not in _CACHE:
        _CACHE["nc"] = _build()
    nc = _CACHE["nc"]

    bff = ml_dtypes.bfloat16
    xs = [np.asarray(t, dtype=np.float32) for t in (x_btc, x_eth, x_sol)]
    # fold v-bias and o-bias: out = attn @ Wo + (sum_j bv_j) @ Wo + bo
    bo2 = (np.asarray(bo, np.float64)
           + np.asarray(bv, np.float64).sum(0) @ np.asarray(Wo, np.float64))
    # weight pack [p, (a, ty, kt, f)]: wpack[p, a,ty,kt,f] = W_ty[a, kt*128+p, f]
    wqkv = np.stack([np.asarray(W, np.float32) for W in (Wq, Wk, Wv)], axis=1)
    wpack = np.ascontiguousarray(
        wqkv.reshape(3, 3, 2, 128, D).transpose(3, 0, 1, 2, 4).reshape(128, 3 * 3 * 2 * D)
    ).astype(bff)
    wo_p = np.ascontiguousarray(
        np.asarray(Wo, np.float32).reshape(2, 128, D).transpose(1, 0, 2).reshape(128, 2 * D)
    ).astype(bff)
    bq_p = np.ascontiguousarray(
        np.asarray(bq, np.float32).reshape(3, 2, 128).transpose(2, 0, 1).reshape(128, 6)
    )
    common = {
        "wpack": wpack,
        "wo": wo_p,
        "bqp": bq_p,
        "bo2": bo2.astype(np.float32).astype(bff),
    }
    in_maps = []
    for c in range(N_CORES):
        b, sq = c // 4, c % 4
        # Roll the sequence so this core's query quarter sits at rows [0:512)
        # (the kernel always projects q from rows 0:512).  k/v see the rolled
        # full sequence, which is fine: softmax+sum over the key axis is
        # permutation-invariant.
        xq = np.stack(
            [np.roll(xs[i][b], -sq * SQ, axis=0) for i in range(3)]
        ).astype(bff)
        in_maps.append({"x": np.ascontiguousarray(xq), **common})
    import os
    res = run_bass_kernel_spmd(
        nc, in_maps, core_ids=list(range(N_CORES)),
        trace=bool(os.environ.get("BASS_TRACE")),
    )
    _CACHE["last_res"] = res

    outs = [np.empty((B, S, D), np.float32) for _ in range(3)]
    for c in range(N_CORES):
        b, sq = c // 4, c % 4
        o = res.results[c]["out"]
        for i in range(3):
            outs[i][b, sq * SQ : (sq + 1) * SQ] = o[i]
    return tuple(outs)


if __name__ == "__main__":
    import reference

    inp = reference.setup_inputs()
    inp = {k: np.asarray(v) for k, v in inp.items()}
    got = kernel(**inp)
    exp = reference.reference(**inp)
    for i in range(3):
        g, e = np.asarray(got[i]), np.asarray(exp[i])
        err = np.abs(g - e).max() / np.abs(e).max()
        print(f"out[{i}] rel err {err:.3e}")


# revision 14
# speedup vs baseline: 1.3913x; 1.0387x over previous
"""Cross-cryptocurrency attention kernel for 8 Trainium2 NeuronCores.

Sharding: 16 (batch, seq-quarter) shards -> core c handles b = c//4,
query rows s in [512*(c%4), 512*(c%4+1)).  Each core computes all 8 heads
and all 9 (query-asset, key-asset) pairs for its query slice, with full
keys/values (S=2048) for its batch, so the output projection is local and
no collectives are needed.

v3 design.  ACT exp (75.5M exps/core -> ~572us incl per-inst overhead) is
the hard floor; everything else is organised to hide under it:
  - host folds biases (bk drops out of softmax exactly; bv/bo fold into
    bo2 = bo + (sum_j bv_j) @ Wo) and pre-packs x/weights in bf16 in the
    exact SBUF layouts, so startup DMAs are few and contiguous.
  - x is transposed by the DMA xbar (dma_start_transpose), zero PE cost.
  - all projections run as [128,256]-output chunk units through a
    dedicated psum bank (psD halves, DVE-memset + start=False chains),
    fully decoupled from the score-psum ring so background work never
    stalls the PE->ACT score pipeline.
  - scores^T[t,s] on PE (lhsT=k^T bf16) in groups (2,3,3,3,3,2); exp on
    ACT PSUM->SBUF bf16.  2-tile groups at the combo boundaries cover the
    next combo's catch-up through the 2-deep score ring.
  - AV with the E tile stationary: out[s,33] accumulates over 16 t-tiles
    at 33 rows each (4x less PE than v-stationary); the ones column of
    [v|1] yields row-sums Z in col 32.  Both combo accumulators pack into
    one psum bank (halves, DVE-memset + start=False).
  - normalize straight off the AV psum: DVE reciprocal + STT mul-add.
  - software pipeline: per combo, the last two AV batches + normalize
    defer into the next combo right after its first score group, so ACT
    never waits at combo boundaries; projection/phase-3 units drip at
    four points per combo with deadlines.
"""

import math
import numpy as np

B = 2
S = 2048
D = 256
H = 8
HD = 32
SQ = 512  # query rows per core
N_CORES = 8
SCALE = 1.0 / math.sqrt(HD)
# 2-tile groups at both ends: boundary exps are 1024 elems, long enough to
# cover the next combo's first score group catching up through the ring.
GROUPS = [(0, 2), (2, 3), (5, 3), (8, 3), (11, 3), (14, 2)]

_CACHE = {}


def _build():
    from contextlib import ExitStack

    import concourse.bass as bass
    import concourse.mybir as mybir
    import concourse.tile as tile
    from concourse import bacc
    from concourse.masks import make_identity

    f32 = mybir.dt.float32
    bf16 = mybir.dt.bfloat16
    AF = mybir.ActivationFunctionType
    ALU = mybir.AluOpType

    nc = bacc.Bacc("TRN2", target_bir_lowering=False, debug=False)

    x_d = nc.dram_tensor("x", [3, S, D], bf16, kind="ExternalInput").ap()
    # host-packed weights: [p, (a, ty q/k/v, kt, f)] bf16 and [p, (a,dt)] f32
    wpack_d = nc.dram_tensor("wpack", [128, 3 * 3 * 2 * D], bf16, kind="ExternalInput").ap()
    wo_d = nc.dram_tensor("wo", [128, 2 * D], bf16, kind="ExternalInput").ap()
    bq_d = nc.dram_tensor("bqp", [128, 6], f32, kind="ExternalInput").ap()
    bo2_d = nc.dram_tensor("bo2", [D], bf16, kind="ExternalInput").ap()
    out_d = nc.dram_tensor("out", [3, SQ, D], f32, kind="ExternalOutput").ap()

    with tile.TileContext(nc) as tc, ExitStack() as ctx:
        const_p = ctx.enter_context(tc.tile_pool(name="const", bufs=1))
        qkv_p = ctx.enter_context(tc.tile_pool(name="qkv", bufs=1))
        acc_p = ctx.enter_context(tc.tile_pool(name="acc", bufs=1))
        e_p = ctx.enter_context(tc.tile_pool(name="epool", bufs=4))
        sm_p = ctx.enter_context(tc.tile_pool(name="small", bufs=2))
        # PSUM: 3+3 score ring + 1 packed AV accumulators + 1 drip bank
        ps_S = ctx.enter_context(tc.tile_pool(name="psS", bufs=2, space="PSUM"))
        psAB_p = ctx.enter_context(tc.tile_pool(name="psAB", bufs=1, space="PSUM"))
        psD_p = ctx.enter_context(tc.tile_pool(name="psD", bufs=1, space="PSUM"))
        psAB = psAB_p.tile([128, 512], f32, name="psAB")
        psD = psD_p.tile([128, 512], f32, name="psD")

        xT = [qkv_p.tile([128, 2 * S], bf16, tag=f"xT{_}", name=f"xT{_}") for _ in range(3)]
        kT = [qkv_p.tile([128, 2 * S], bf16, tag=f"kT{_}", name=f"kT{_}") for _ in range(3)]
        qT = [qkv_p.tile([128, 2 * SQ], bf16, tag=f"qT{_}", name=f"qT{_}") for _ in range(3)]
        v1 = [qkv_p.tile([128, 16 * (H * 33)], bf16, tag=f"v1_{_}", name=f"v1_{_}") for _ in range(3)]
        out_acc = [acc_p.tile([128, 4 * D], f32, tag=f"oacc{_}", name=f"oacc{_}") for _ in range(3)]

        def dmaT_unit(a, c):
            def run():
                nc.sync.dma_start_transpose(
                    xT[a].rearrange("p (dt s) -> p dt s", dt=2)[:, :, c * 512 : (c + 1) * 512],
                    x_d[a][c * 512 : (c + 1) * 512, :],
                )
            return run

        # ---- startup DMAs ordered for minimum time-to-first-score-group:
        # x chunk 0, tiny bq, asset-0 weights; everything else after ----
        dmaT_unit(0, 0)()
        bq_sb = const_p.tile([128, 6], f32)
        nc.sync.dma_start(bq_sb[:], bq_d)
        wsb = const_p.tile([128, 3 * 3 * 2 * D], bf16)
        nc.sync.dma_start(wsb[:, 0 : 3 * 2 * D], wpack_d[:, 0 : 3 * 2 * D])
        for c in range(1, 4):
            dmaT_unit(0, c)()
        nc.sync.dma_start(wsb[:, 3 * 2 * D :], wpack_d[:, 3 * 2 * D :])
        wo_sb = const_p.tile([128, 2 * D], bf16)
        nc.sync.dma_start(wo_sb[:], wo_d)
        bo2_row = const_p.tile([1, D], bf16)
        nc.sync.dma_start(bo2_row[:], bo2_d[None, :])
        ident = const_p.tile([128, 128], f32)
        make_identity(nc, ident[:])
        onesb = const_p.tile([1, 128], bf16)
        nc.gpsimd.memset(onesb[:], 1.0)

        # ---- drip bank: [128,256] halves.  Chains open with start=True:
        # the bank-wide zero region is applied lazily (zero-on-next-matmul-
        # touch), so the other half's finished result stays readable for its
        # pending DVE copy; chains themselves are sequential in PE order. ----
        dctr = [0]

        def dhalf():
            hh = dctr[0] & 1
            dctr[0] += 1
            return psD[:, hh * 256 : (hh + 1) * 256]

        WT_Q, WT_K, WT_V = 0, 1, 2

        def wcol(a, ty, kt, off, width):
            base = a * (3 * 2 * D) + ty * (2 * D) + kt * D + off
            return wsb[:, base : base + width]

        def k_unit(a, dt, n):  # kT[a] cols [dt*S + 256n, +256)
            def run():
                reg = dhalf()
                for kt in range(2):
                    nc.tensor.matmul(
                        reg,
                        wcol(a, WT_K, kt, dt * 128, 128),
                        xT[a][:, kt * S + n * 256 : kt * S + (n + 1) * 256],
                        start=(kt == 0), stop=(kt == 1), skip_group_check=True,
                    )
                nc.vector.tensor_copy(
                    kT[a][:, dt * S + n * 256 : dt * S + (n + 1) * 256], reg
                )
            return run

        def q_unit(a, dt, n):  # qT[a] cols [dt*SQ + 256n, +256)
            def run():
                reg = dhalf()
                for kt in range(2):
                    nc.tensor.matmul(
                        reg,
                        wcol(a, WT_Q, kt, dt * 128, 128),
                        xT[a][:, kt * S + n * 256 : kt * S + (n + 1) * 256],
                        start=(kt == 0), stop=(kt == 1), skip_group_check=True,
                    )
                nc.vector.tensor_scalar_add(
                    qT[a][:, dt * SQ + n * 256 : dt * SQ + (n + 1) * 256],
                    reg,
                    bq_sb[:, a * 2 + dt : a * 2 + dt + 1],
                )
            return run

        def v_unit(a, st):
            def run():
                reg = dhalf()
                for kt in range(2):
                    nc.tensor.matmul(
                        reg,
                        xT[a][:, kt * S + st * 128 : kt * S + (st + 1) * 128],
                        wcol(a, WT_V, kt, 0, D),
                        start=(kt == 0), stop=(kt == 1), skip_group_check=True,
                    )
                dst = v1[a][
                    :, st * (H * 33) : (st + 1) * (H * 33)
                ].rearrange("p (h x) -> p h x", x=33)[:, :, 0:32]
                nc.vector.tensor_copy(dst, reg.rearrange("p (h x) -> p h x", x=32))
            return run

        def ones_unit(a):
            def run():
                nc.gpsimd.memset(
                    v1[a].rearrange("p (t h x) -> p (t h) x", h=H, x=33)[:, :, 32:33],
                    1.0,
                )
            return run

        # ======== Phase 2: one (i, j, h) combo ========
        def emit_av(eg, t0, glen, j, h, reg):
            for u in range(glen):
                tt = t0 + u
                for k in range(4):
                    nc.tensor.matmul(
                        reg[:, k * 33 : (k + 1) * 33],
                        eg[:, u * 512 + k * 128 : u * 512 + (k + 1) * 128],
                        v1[j][:, tt * (H * 33) + h * 33 : tt * (H * 33) + (h + 1) * 33],
                        start=False,
                        stop=(tt == 15 and k == 3),
                        skip_group_check=True,
                    )

        def norm_unit(i, j, h, reg):
            def run():
                rr4 = sm_p.tile([128, 4], f32, tag="rr", name="rr")
                nc.vector.reciprocal_approx_fast(
                    rr4[:],
                    reg.rearrange("p (k x) -> p k x", x=33)[:, :, 32],
                )
                for k in range(4):
                    oa = out_acc[i][:, k * D + h * 32 : k * D + (h + 1) * 32]
                    src = reg[:, k * 33 : k * 33 + 32]
                    if j == 0:
                        nc.vector.tensor_scalar_mul(oa, src, rr4[:, k : k + 1])
                    else:
                        nc.vector.scalar_tensor_tensor(
                            oa, src, rr4[:, k : k + 1], oa, op0=ALU.mult, op1=ALU.add
                        )
            return run

        def combo(ci, i, j, h, tail, drip, pre_sc=None, pre_av=None, pre_tail=None):
            hp = 32 * (h % 4)
            hc = h // 4
            reg = psAB[:, (ci % 2) * 256 : (ci % 2) * 256 + 132]
            nc.vector.memset(reg, 0.0)
            egs = []

            def sc(gi):
                t0, glen = GROUPS[gi]
                psS = ps_S.tile([128, glen * 512], f32, tag="psS", name="ps2")
                for u in range(glen):
                    tt = t0 + u
                    nc.tensor.matmul(
                        psS[:, u * 512 : (u + 1) * 512],
                        kT[j][hp : hp + 32, hc * S + tt * 128 : hc * S + (tt + 1) * 128],
                        qT[i][hp : hp + 32, hc * SQ : (hc + 1) * SQ],
                        start=True,
                        stop=True,
                        tile_position=(hp, 0),
                    )
                eg = e_p.tile([128, 3 * 512], bf16, tag="eg", name="eg")
                nc.scalar.activation(eg[:, 0 : glen * 512], psS[:], AF.Exp, scale=SCALE)
                egs.append((eg, t0, glen))

            def hook(d, gi):
                if d and gi in d:
                    for u in d[gi]:
                        u()

            hook(pre_sc, 0)
            sc(0)
            if pre_tail:
                for u in pre_tail:
                    u()
            for t in tail:
                t()
            hook(pre_sc, 1)
            drip(1)
            sc(1)
            hook(pre_av, 0)
            drip(1)
            emit_av(*egs[0], j, h, reg)
            hook(pre_sc, 2)
            drip(1)
            sc(2)
            hook(pre_av, 1)
            drip(1)
            emit_av(*egs[1], j, h, reg)
            hook(pre_sc, 3)
            drip(1)
            sc(3)
            hook(pre_av, 2)
            drip(1)
            emit_av(*egs[2], j, h, reg)
            hook(pre_sc, 4)
            drip(1)
            sc(4)
            hook(pre_av, 3)
            drip(1)
            emit_av(*egs[3], j, h, reg)
            hook(pre_sc, 5)
            drip(1)
            sc(5)
            return [
                lambda: emit_av(*egs[4], j, h, reg),
                lambda: emit_av(*egs[5], j, h, reg),
                norm_unit(i, j, h, reg),
            ]

        # ======== Phase 3: output projection for one asset, as units ======
        aT = [acc_p.tile([128, 2 * SQ], bf16, tag=f"aT{_}", name=f"aT{_}") for _ in range(3)]

        def t_unit(i, dt, half):  # transpose out_acc block -> aT bf16
            def run():
                reg = dhalf()
                for k in range(2):
                    st = 2 * half + k
                    nc.tensor.matmul(
                        reg[:, k * 128 : (k + 1) * 128],
                        out_acc[i][:, st * D + dt * 128 : st * D + dt * 128 + 128],
                        ident[:],
                        is_transpose=True,
                        start=(k == 0), stop=True, skip_group_check=True,
                    )
                nc.vector.tensor_copy(
                    aT[i][:, dt * SQ + half * 256 : dt * SQ + (half + 1) * 256], reg
                )
            return run

        def p_unit(i, st):
            def run():
                reg = dhalf()
                for dt in range(2):
                    nc.tensor.matmul(
                        reg,
                        aT[i][:, dt * SQ + st * 128 : dt * SQ + (st + 1) * 128],
                        wo_sb[:, dt * D : (dt + 1) * D],
                        start=(dt == 0), stop=False, skip_group_check=True,
                    )
                nc.tensor.matmul(
                    reg,
                    onesb[0:1, 0:128],
                    bo2_row[0:1, :],
                    start=False, stop=True, skip_group_check=True,
                )
                ot = sm_p.tile([128, D], f32, tag="ot", name="ot")
                nc.vector.tensor_copy(ot[:], reg)
                nc.sync.dma_start(
                    out_d[i].rearrange("(st p) d -> st p d", p=128)[st], ot[:]
                )
            return run

        def ph3_units(i):
            return [t_unit(i, dt, half) for dt in range(2) for half in range(2)] + [
                p_unit(i, st) for st in range(4)
            ]

        # ======== Emission schedule ========
        # startup prefix: just enough for combo (0,0,0) group 0
        q_unit(0, 0, 0)()
        q_unit(0, 0, 1)()
        k_unit(0, 0, 0)()
        ones_unit(0)()

        # combo-0/1 custom placement: asset-0 dt0 k-units and v-units land
        # exactly where the score groups / AV batches need them
        c0_pre_sc = {
            1: [k_unit(0, 0, 1), k_unit(0, 0, 2)],
            2: [k_unit(0, 0, 3)],
            3: [k_unit(0, 0, 4), k_unit(0, 0, 5)],
            4: [k_unit(0, 0, 6)],
            5: [k_unit(0, 0, 7)],
        }
        c0_pre_av = {
            0: [v_unit(0, 0), v_unit(0, 1)],
            1: [v_unit(0, 2), v_unit(0, 3), v_unit(0, 4)],
            2: [v_unit(0, 5), v_unit(0, 6), v_unit(0, 7)],
            3: [v_unit(0, 8), v_unit(0, 9), v_unit(0, 10)],
        }
        c1_pre_tail = [v_unit(0, st) for st in range(11, 16)]

        # deadline-tagged drip queue
        dripq = []
        for dt in range(2):
            for n in range(2):
                dripq.append((q_unit(0, 1, n), 0))
        for n in range(8):
            dripq.append((k_unit(0, 1, n), 0))
        for a in (1, 2):
            for c in range(4):
                dripq.append((dmaT_unit(a, c), 0))
            dripq.append((ones_unit(a), 0))
            for dt in range(2):
                for n in range(2):
                    dripq.append((q_unit(a, dt, n), 0))
            for dt in range(2):
                for n in range(8):
                    dripq.append((k_unit(a, dt, n), 0))
            for st in range(16):
                dripq.append((v_unit(a, st), 0))
        for u in ph3_units(0):
            dripq.append((u, 24))
        for u in ph3_units(1):
            dripq.append((u, 48))

        ci_box = [0]

        def drip(budget):
            while dripq and budget > 0 and dripq[0][1] <= ci_box[0]:
                dripq.pop(0)[0]()
                budget -= 1

        tail = []
        ci = 0
        for i in range(3):
            for j in range(3):
                for h in range(H):
                    ci_box[0] = ci
                    combo_kw = {}
                    if ci == 0:
                        combo_kw = dict(pre_sc=c0_pre_sc, pre_av=c0_pre_av)
                    elif ci == 1:
                        combo_kw = dict(pre_tail=c1_pre_tail)
                    tail = combo(ci, i, j, h, tail, drip, **combo_kw)
                    ci += 1
        for t in tail:
            t()
        while dripq:
            dripq.pop(0)[0]()
        for u in ph3_units(2):
            u()
    nc.compile()
    return nc


def kernel(x_btc, x_eth, x_sol, Wq, bq, Wk, bk, Wv, bv, Wo, bo):
    import ml_dtypes
    from concourse.bass_utils import run_bass_kernel_spmd

    if "nc" not in _CACHE:
        _CACHE["nc"] = _build()
    nc = _CACHE["nc"]

    bff = ml_dtypes.bfloat16
    xs = [np.asarray(t, dtype=np.float32) for t in (x_btc, x_eth, x_sol)]
    # fold v-bias and o-bias: out = attn @ Wo + (sum_j bv_j) @ Wo + bo
    bo2 = (np.asarray(bo, np.float64)
           + np.asarray(bv, np.float64).sum(0) @ np.asarray(Wo, np.float64))
    # weight pack [p, (a, ty, kt, f)]: wpack[p, a,ty,kt,f] = W_ty[a, kt*128+p, f]
    wqkv = np.stack([np.asarray(W, np.float32) for W in (Wq, Wk, Wv)], axis=1)
    wpack = np.ascontiguousarray(
        wqkv.reshape(3, 3, 2, 128, D).transpose(3, 0, 1, 2, 4).reshape(128, 3 * 3 * 2 * D)
    ).astype(bff)
    wo_p = np.ascontiguousarray(
        np.asarray(Wo, np.float32).reshape(2, 128, D).transpose(1, 0, 2).reshape(128, 2 * D)
    ).astype(bff)
    bq_p = np.ascontiguousarray(
        np.asarray(bq, np.float32).reshape(3, 2, 128).transpose(2, 0, 1).reshape(128, 6)
    )
    common = {
        "wpack": wpack,
        "wo": wo_p,
        "bqp": bq_p,
        "bo2": bo2.astype(np.float32).astype(bff),
    }
    in_maps = []
    for c in range(N_CORES):
        b, sq = c // 4, c % 4
        # Roll the sequence so this core's query quarter sits at rows [0:512)
        # (the kernel always projects q from rows 0:512).  k/v see the rolled
        # full sequence, which is fine: softmax+sum over the key axis is
        # permutation-invariant.
        xq = np.stack(
            [np.roll(xs[i][b], -sq * SQ, axis=0) for i in range(3)]
        ).astype(bff)
        in_maps.append({"x": np.ascontiguousarray(xq), **common})
    import os
    res = run_bass_kernel_spmd(
        nc, in_maps, core_ids=list(range(N_CORES)),
        trace=bool(os.environ.get("BASS_TRACE")),
    )
    _CACHE["last_res"] = res

    outs = [np.empty((B, S, D), np.float32) for _ in range(3)]
    for c in range(N_CORES):
        b, sq = c // 4, c % 4
        o = res.results[c]["out"]
        for i in range(3):
            outs[i][b, sq * SQ : (sq + 1) * SQ] = o[i]
    return tuple(outs)


if __name__ == "__main__":
    import reference

    inp = reference.setup_inputs()
    inp = {k: np.asarray(v) for k, v in inp.items()}
    got = kernel(**inp)
    exp = reference.reference(**inp)
    for i in range(3):
        g, e = np.asarray(got[i]), np.asarray(exp[i])
        err = np.abs(g - e).max() / np.abs(e).max()
        print(f"out[{i}] rel err {err:.3e}")
